# revision 8
# baseline (speedup 1.0000x reference)
"""Trainium2 Bass kernel for nn_DeformBasicBlock1 (deformable conv block).

Fully fused single-invocation SPMD program over 8 cores, group-sharded:
core g owns channel-group g (8 x-channels / 81 offset channels / 8 output
channels).  Cross-core exchange happens on device:
  AllGather(x slices) -> conv1 -> deform1 -> ReduceScatter(partials)
  -> per-channel BN1+relu -> AllGather -> conv2 -> deform2
  -> ReduceScatter -> BN2 + residual + relu -> per-core output slice.
The jitted executable is built once and cached in module globals; repeat
kernel() calls only move input/output slices (~35 MB) over the wire.
"""
import json
import numpy as np

import jax
import jax.numpy as jnp
from jax.sharding import Mesh, PartitionSpec, NamedSharding
from jax.experimental.shard_map import shard_map

import concourse.bass as bass
import concourse.mybir as mybir
from concourse.tile import TileContext
import concourse.bass_utils as bass_utils
import concourse.tile_utils as tile_utils

# ---------------------------------------------------------------- tilefix --
_orig_compile_bir_kernel = bass_utils.compile_bir_kernel


def _split_waits_json(bir_json: bytes) -> bytes:
    j = json.loads(bir_json)
    ctr = 0
    changed = False
    for f in j["functions"]:
        for b in f["blocks"]:
            insts = b["instructions"]
            if not any(
                len((i.get("sync_info") or {}).get("on_wait") or []) > 1
                for i in insts
            ):
                continue
            changed = True
            out = []
            for inst in insts:
                si = inst.get("sync_info")
                if si:
                    ow = si.get("on_wait") or []
                    if len(ow) > 1:
                        for w in ow[:-1]:
                            ctr += 1
                            nop = {
                                "engine": inst["engine"],
                                "ins": [],
                                "outs": [],
                                "name": f"WSPLIT-{ctr}",
                                "opcode": "NoOp",
                                "sync_info": {"on_update": [], "on_wait": [w]},
                            }
                            if "debug" in inst:
                                nop["debug"] = inst["debug"]
                            out.append(nop)
                        si["on_wait"] = [ow[-1]]
                out.append(inst)
            b["instructions"] = out
    return json.dumps(j).encode() if changed else bir_json


def _patched_compile_bir_kernel(bir_json, tmpdir, neff_name="file.neff"):
    if isinstance(bir_json, str):
        bir_json = bir_json.encode()
    return _orig_compile_bir_kernel(_split_waits_json(bir_json), tmpdir, neff_name)


bass_utils.compile_bir_kernel = _patched_compile_bir_kernel
import concourse.bass2jax as _b2j  # noqa: E402

_b2j.compile_bir_kernel = _patched_compile_bir_kernel
try:
    tile_utils.max_sbuf_usage = 204 * 1024
except Exception:
    pass

# ------------------------------------------------------------- constants --
B, D, H, W = 2, 8, 56, 56
CPG, G, K = 8, 8, 27
OCG = 81
V = D * H * W
BV = B * V
PLANE = 3364  # 58*58
NB, BH = 14, 4
P = NB * CPG  # 112
CH = D * BH * W  # 1792
XD, XH, XWW = 14, 10, 62
XSZ = XD * XH * XWW
XVOL = XD * 62 * 62
SS = 5
CLAMP = 1.999
F32 = mybir.dt.float32
F16 = mybir.dt.float16
AX = mybir.AxisListType
ALU = mybir.AluOpType
ACTF = mybir.ActivationFunctionType
NCORES = 8
RG = [list(range(NCORES))]

# packed-input blob layout (per core)
XS_OFF = 0
WT1_OFF = CPG * BV            # 401408
WT2_OFF = WT1_OFF + 64 * K * OCG  # + 139968
N16 = WT2_OFF + 64 * K * OCG  # 681344 f16 elements
BO1_OFF = 0
BO2_OFF = OCG
WD1_OFF = 2 * OCG
WD2_OFF = WD1_OFF + CPG * K * 64
GB_OFF = WD2_OFF + CPG * K * 64
N32 = GB_OFF + CPG * 4


def mkap(tile, off, dims):
    ap = tile[:]
    return bass.AP(tensor=ap.tensor, offset=ap.offset + off,
                   ap=[list(ap.ap[0])] + [list(d) for d in dims])


def dmkap(t_ap, off, dims):
    return bass.AP(tensor=t_ap.tensor, offset=t_ap.offset + off,
                   ap=[list(d) for d in dims])


ZBLK = 8192


def zero_dram(nc, zero_sb, dram_ap, rows, total):
    # stride-0 repeat DMAs corrupt data on this DMA engine; use one DMA per
    # block from a real zero tile instead.
    nblk = total // ZBLK
    rem = total - nblk * ZBLK
    for i in range(nblk):
        nc.sync.dma_start(out=dmkap(dram_ap, i * ZBLK, [[total, rows], [1, ZBLK]]),
                          in_=dmkap(zero_sb[:], 0, [[ZBLK, rows], [1, ZBLK]]))
    if rem:
        nc.sync.dma_start(out=dmkap(dram_ap, nblk * ZBLK, [[total, rows], [1, rem]]),
                          in_=dmkap(zero_sb[:], 0, [[ZBLK, rows], [1, rem]]))


def conv_phase(nc, tc, sfx, xpad_dram, wt_sb, bias_sb, off_dram):
    """27-tap conv: xpad_dram [64, B*10*PLANE] -> off_dram [81, B*D*3136]."""
    GUARD = 64
    CHUNKS = []
    for r0 in range(0, 58, 8):
        nr = min(8, 58 - r0)
        v0 = max(1, r0)
        v1 = min(57, r0 + nr)
        CHUNKS.append((r0 * 58, nr * 58, v0 - r0, v1 - v0))
    with tc.tile_pool(name=f"convp{sfx}", bufs=2) as pool, \
         tc.tile_pool(name=f"convps{sfx}", bufs=4, space="PSUM") as pspool:
        for b in range(B):
            for j in range(4):
                xpc = pool.tile([64, 2 * GUARD + 4 * PLANE], F32, tag="xpc")
                nc.vector.memset(xpc[:, :GUARD], 0.0)
                nc.vector.memset(xpc[:, GUARD + 4 * PLANE:], 0.0)
                nc.sync.dma_start(
                    out=xpc[:, GUARD:GUARD + 4 * PLANE],
                    in_=dmkap(xpad_dram[:], (b * 10 + 2 * j) * PLANE,
                              [[B * 10 * PLANE, 64], [1, 4 * PLANE]]))
                for ds in range(2):
                    d = 2 * j + ds
                    for (n0, nsz, vr, nv) in CHUNKS:
                        ps = pspool.tile([OCG, 512], F32, tag="cps")
                        for k in range(K):
                            kd, kh, kw = k // 9, (k // 3) % 3, k % 3
                            roff = GUARD + (ds + kd) * PLANE + (kh - 1) * 58 + (kw - 1) + n0
                            nc.tensor.matmul(ps[:, :nsz], wt_sb[:, k, :],
                                             mkap(xpc, roff, [[1, nsz]]),
                                             start=(k == 0), stop=(k == K - 1))
                        ot = pool.tile([OCG, 512], F32, tag="convot")
                        nc.vector.tensor_tensor(
                            out=ot[:, :nsz], in0=ps[:, :nsz],
                            in1=mkap(bias_sb, 0, [[0, nsz]]), op=ALU.add)
                        if nv <= 0:
                            continue
                        real_r0 = n0 // 58 + vr - 1
                        nc.sync.dma_start(
                            out=dmkap(off_dram[:], (b * D + d) * 3136 + real_r0 * 56,
                                      [[B * D * 3136, OCG], [1, nv * 56]]),
                            in_=mkap(ot, vr * 58 + 1, [[58, nv], [1, 56]]))


def dense_phase(nc, tc, sfx, xw_dram, off_dram, wd_sb, partial_dram, colsd_dram):
    """Dense 5^3 deform + einsum -> partial_dram [64, BV] (band-perm)."""
    with tc.tile_pool(name=f"densep{sfx}", bufs=1) as pool, \
         tc.tile_pool(name=f"densew{sfx}", bufs=1) as wpool, \
         tc.tile_pool(name=f"denseps{sfx}", bufs=2, space="PSUM") as pspool:
        for b in range(B):
            xw = pool.tile([P, XSZ], F32, tag="xw")
            for dd in range(XD):
                nc.sync.dma_start(
                    out=mkap(xw, dd * XH * XWW, [[1, 620]]),
                    in_=dmkap(xw_dram[:], b * XVOL + dd * 62 * 62,
                              [[BH * XWW, NB], [B * XVOL, CPG], [1, XH * XWW]]))
            for k in range(K):
                kd, kh, kw = k // 9 - 1, (k // 3) % 3 - 1, k % 3 - 1
                offt = pool.tile([P, 3, CH], F32, tag="offt")
                for ax in range(3):
                    for dd in range(D):
                        nc.sync.dma_start(
                            out=mkap(offt, ax * CH + dd * BH * W, [[1, BH * W]]),
                            in_=dmkap(off_dram[:],
                                      (3 * k + ax) * B * D * 3136 + (b * D + dd) * 3136,
                                      [[BH * W, NB], [0, CPG], [1, BH * W]]))
                nc.vector.tensor_scalar(out=offt[:], in0=offt[:], scalar1=CLAMP,
                                        scalar2=-CLAMP, op0=ALU.min, op1=ALU.max)
                hw = pool.tile([P, SS, CH], F32, tag="hw")
                for a in range(SS):
                    nc.scalar.activation(hw[:, a, :], offt[:, 2, :], ACTF.Abs,
                                         bias=float(-(a - 2)), scale=1.0)
                    nc.scalar.activation(hw[:, a, :], hw[:, a, :], ACTF.Relu,
                                         bias=1.0, scale=-1.0)
                cols = wpool.tile([P, CH], F32, tag="cols")
                pt = wpool.tile([P, CH], F32, tag="pt")
                at = wpool.tile([P, CH], F32, tag="at")
                tt = wpool.tile([P, CH], F32, tag="tt")
                hdsl = pool.tile([P, CH], F32, tag="hdsl")
                hhsl = pool.tile([P, CH], F32, tag="hhsl")
                first = True
                for sd in range(SS):
                    nc.scalar.activation(hdsl[:], offt[:, 0, :], ACTF.Abs,
                                         bias=float(-(sd - 2)), scale=1.0)
                    nc.scalar.activation(hdsl[:], hdsl[:], ACTF.Relu,
                                         bias=1.0, scale=-1.0)
                    for sh in range(SS):
                        nc.scalar.activation(hhsl[:], offt[:, 1, :], ACTF.Abs,
                                             bias=float(-(sh - 2)), scale=1.0)
                        nc.scalar.activation(hhsl[:], hhsl[:], ACTF.Relu,
                                             bias=1.0, scale=-1.0)
                        nc.vector.tensor_tensor(out=pt[:], in0=hdsl[:],
                                                in1=hhsl[:], op=ALU.mult)
                        for sw in range(SS):
                            xoff = ((1 + kd + sd) * XH * XWW + (1 + kh + sh) * XWW
                                    + (1 + kw + sw))
                            xap = mkap(xw, xoff, [[XH * XWW, D], [XWW, BH], [1, W]])
                            dst = at if sw == 0 else tt
                            nc.vector.tensor_tensor(out=dst[:], in0=xap,
                                                    in1=hw[:, sw, :], op=ALU.mult)
                            if sw > 0:
                                nc.vector.tensor_tensor(out=at[:], in0=at[:],
                                                        in1=tt[:], op=ALU.add)
                        if first:
                            nc.vector.tensor_tensor(out=cols[:], in0=pt[:], in1=at[:],
                                                    op=ALU.mult)
                            first = False
                        else:
                            nc.gpsimd.tensor_tensor(out=tt[:], in0=pt[:], in1=at[:],
                                                    op=ALU.mult)
                            nc.gpsimd.tensor_tensor(out=cols[:], in0=cols[:], in1=tt[:],
                                                    op=ALU.add)
                nc.sync.dma_start(
                    out=dmkap(colsd_dram[:], (b * K + k) * CH,
                              [[B * K * CH, P], [1, CH]]),
                    in_=cols[:])
            tc.strict_bb_all_engine_barrier()
            for hb in range(NB):
                ps2 = pspool.tile([64, 2048], F32, tag="eps")
                for k in range(K):
                    cr = wpool.tile([CPG, CH], F32, tag="colsr")
                    nc.sync.dma_start(
                        out=cr[:],
                        in_=dmkap(colsd_dram[:], hb * CPG * B * K * CH + (b * K + k) * CH,
                                  [[B * K * CH, CPG], [1, CH]]))
                    for i in range(4):
                        nc.tensor.matmul(ps2[:, i * 512:i * 512 + 448], wd_sb[:, k, :],
                                         cr[:, i * 448:(i + 1) * 448],
                                         start=(k == 0), stop=(k == K - 1))
                pot = wpool.tile([64, CH], F32, tag="pot")
                nc.vector.tensor_copy(out=pot[:], in_=mkap(ps2, 0, [[512, 4], [1, 448]]))
                nc.sync.dma_start(
                    out=dmkap(partial_dram[:], b * V + hb * CH, [[BV, 64], [1, CH]]),
                    in_=pot[:])


def ensure_consts(nc):
    for v in (2.0, -2.0, -1.0, 1e-5):
        key = (F32, v)
        if key not in nc.const_aps.aps:
            t = nc.alloc_sbuf_tensor(f"const-f32-{v}", [128, 1], F32)
            nc.gpsimd.memset(t.ap(), v)
            nc.const_aps.aps[key] = t.ap()


def bn_stats8(nc, tc, pool, sfx, src_dram, gamma_sb, beta_sb):
    """BN scale/shift for [CPG, BV] shard (band layout is irrelevant)."""
    sum_t = pool.tile([CPG, 1], F32, tag=f"bnsum{sfx}")
    sq_t = pool.tile([CPG, 1], F32, tag=f"bnsq{sfx}")
    t1 = pool.tile([CPG, 1], F32, tag=f"bnt1{sfx}")
    t2 = pool.tile([CPG, 1], F32, tag=f"bnt2{sfx}")
    with tc.tile_pool(name=f"bnstat{sfx}", bufs=1) as big:
        for i in range(NCHK):
            ht = big.tile([CPG, CSZ], F32, tag=f"bnh{sfx}")
            sqv = big.tile([CPG, CSZ], F32, tag=f"bnsqv{sfx}")
            nc.sync.dma_start(out=ht[:], in_=dmkap(src_dram, i * CSZ, [[BV, CPG], [1, CSZ]]))
            nc.vector.tensor_reduce(out=t1[:], in_=ht[:], axis=AX.X, op=ALU.add)
            nc.vector.tensor_tensor(out=sqv[:], in0=ht[:], in1=ht[:], op=ALU.mult)
            nc.vector.tensor_reduce(out=t2[:], in_=sqv[:], axis=AX.X, op=ALU.add)
            if i == 0:
                nc.vector.tensor_copy(out=sum_t[:], in_=t1[:])
                nc.vector.tensor_copy(out=sq_t[:], in_=t2[:])
            else:
                nc.vector.tensor_tensor(out=sum_t[:], in0=sum_t[:], in1=t1[:], op=ALU.add)
                nc.vector.tensor_tensor(out=sq_t[:], in0=sq_t[:], in1=t2[:], op=ALU.add)
    N = float(BV)
    scale = pool.tile([CPG, 1], F32, tag=f"bnscale{sfx}")
    shift = pool.tile([CPG, 1], F32, tag=f"bnshift{sfx}")
    mean = t1
    nc.vector.tensor_scalar(out=mean[:], in0=sum_t[:], scalar1=1.0 / N, scalar2=0.0,
                            op0=ALU.mult, op1=ALU.add)
    var = t2
    nc.vector.tensor_scalar(out=var[:], in0=sq_t[:], scalar1=1.0 / N, scalar2=0.0,
                            op0=ALU.mult, op1=ALU.add)
    msq = pool.tile([CPG, 1], F32, tag=f"bnmsq{sfx}")
    nc.vector.tensor_tensor(out=msq[:], in0=mean[:], in1=mean[:], op=ALU.mult)
    nc.vector.tensor_tensor(out=var[:], in0=var[:], in1=msq[:], op=ALU.subtract)
    rstd = pool.tile([CPG, 1], F32, tag=f"bnrstd{sfx}")
    nc.scalar.activation(out=rstd[:], in_=var[:], func=ACTF.Sqrt, bias=1e-5, scale=1.0)
    nc.vector.reciprocal(out=rstd[:], in_=rstd[:])
    nc.vector.tensor_tensor(out=scale[:], in0=gamma_sb[:], in1=rstd[:], op=ALU.mult)
    nc.vector.tensor_tensor(out=shift[:], in0=mean[:], in1=scale[:], op=ALU.mult)
    nc.vector.tensor_tensor(out=shift[:], in0=beta_sb[:], in1=shift[:], op=ALU.subtract)
    return scale, shift


NCHK = 4
CSZ = BV // NCHK


def deband_store(nc, src_tile, dst_dram, i):
    """Store band-layout chunk i of [CPG, CSZ] to canonical [CPG, BV] DRAM."""
    b, half = i // 2, i % 2
    for hbr in range(7):
        hb = half * 7 + hbr
        nc.sync.dma_start(
            out=dmkap(dst_dram, b * V + hb * BH * W, [[BV, CPG], [3136, D], [1, BH * W]]),
            in_=mkap(src_tile, hbr * CH, [[BH * W, D], [1, BH * W]]))


def band_load(nc, dst_tile, src_dram, i):
    """Load canonical [CPG, BV] DRAM into band-layout chunk i [CPG, CSZ]."""
    b, half = i // 2, i % 2
    for hbr in range(7):
        hb = half * 7 + hbr
        nc.sync.dma_start(
            out=mkap(dst_tile, hbr * CH, [[BH * W, D], [1, BH * W]]),
            in_=dmkap(src_dram, b * V + hb * BH * W, [[BV, CPG], [3136, D], [1, BH * W]]))


# ------------------------------------------------------------ program nc --
def build_fused(debug=False):
    nc = bass.Bass("TRN2", target_bir_lowering=False, num_devices=NCORES)
    ensure_consts(nc)
    pk16_in = nc.declare_dram_parameter("pk16", [1, N16], F16, isOutput=False)
    pk32_in = nc.declare_dram_parameter("pk32", [1, N32], F32, isOutput=False)
    out_d = nc.declare_dram_parameter("out", [CPG, BV], F16, isOutput=True)

    xs32_d = nc.dram_tensor("xs32", [CPG, BV], F32)
    xb16_d = nc.dram_tensor("xb16", [CPG, BV], F16)
    xfull16_d = nc.dram_tensor("xfull16", [64, BV], F16, addr_space="Shared")
    xfull_d = nc.dram_tensor("xfull", [64, BV], F32)
    xpad_d = nc.dram_tensor("xpad", [64, B * 10 * PLANE], F32)
    xw_d = nc.dram_tensor("xw", [CPG, B * XVOL], F32)
    xw2_d = nc.dram_tensor("xw2", [CPG, B * XVOL], F32)
    hpad_d = nc.dram_tensor("hpad", [64, B * 10 * PLANE], F32)
    off_d = nc.dram_tensor("offs", [OCG, B * D * 3136], F32)
    colsd_d = nc.dram_tensor("colsd", [P, B * K * CH], F32)
    part_d = nc.dram_tensor("part", [64, BV], F32)
    h1s_d = nc.dram_tensor("h1s", [CPG, BV], F32)
    h2s_d = nc.dram_tensor("h2s", [CPG, BV], F32)
    hg1_d = nc.dram_tensor("hg1", [CPG, BV], F32)
    hfull_d = nc.dram_tensor("hfull", [64, BV], F32, addr_space="Shared")

    dbg = {}
    if debug:
        for nm, shp in (("dxfull", [64, BV]), ("doff1", [OCG, BV]),
                        ("dpart1", [64, BV]), ("dh1s", [CPG, BV]),
                        ("dhg1", [CPG, BV]), ("dhfull", [64, BV]),
                        ("doff2", [OCG, BV]), ("dpart2", [64, BV]),
                        ("dh2s", [CPG, BV]), ("dxpad", [64, B * 10 * PLANE]),
                        ("dxw", [CPG, B * XVOL])):
            dbg[nm] = nc.declare_dram_parameter(nm, shp, F32, isOutput=True)

    def dump(nm, src, rows, total):
        if not debug:
            return
        nc.sync.dma_start(out=dmkap(dbg[nm][:], 0, [[total, rows], [1, total]]),
                          in_=dmkap(src[:], 0, [[total, rows], [1, total]]))

    with TileContext(nc) as tc:
        with tc.tile_pool(name="single", bufs=1) as sp:
            wt1_sb = sp.tile([64, K, OCG], F32, tag="wt1")
            wt2_sb = sp.tile([64, K, OCG], F32, tag="wt2")
            for woff, wsb, wtag in ((WT1_OFF, wt1_sb, "w16a"), (WT2_OFF, wt2_sb, "w16b")):
                w16 = sp.tile([64, K * OCG], F16, tag=wtag)
                nc.sync.dma_start(
                    out=w16[:],
                    in_=dmkap(pk16_in[:], woff, [[K * OCG, 64], [1, K * OCG]]))
                nc.vector.tensor_copy(out=wsb[:], in_=w16[:])
            bo1_sb = sp.tile([OCG, 1], F32, tag="bo1")
            nc.sync.dma_start(out=bo1_sb[:],
                              in_=dmkap(pk32_in[:], BO1_OFF, [[1, OCG], [1, 1]]))
            bo2_sb = sp.tile([OCG, 1], F32, tag="bo2")
            nc.sync.dma_start(out=bo2_sb[:],
                              in_=dmkap(pk32_in[:], BO2_OFF, [[1, OCG], [1, 1]]))
            wd1_sb = sp.tile([CPG, K, 64], F32, tag="wd1")
            nc.sync.dma_start(out=wd1_sb[:],
                              in_=dmkap(pk32_in[:], WD1_OFF, [[K * 64, CPG], [1, K * 64]]))
            wd2_sb = sp.tile([CPG, K, 64], F32, tag="wd2")
            nc.sync.dma_start(out=wd2_sb[:],
                              in_=dmkap(pk32_in[:], WD2_OFF, [[K * 64, CPG], [1, K * 64]]))
            gb_sb = sp.tile([CPG, 4], F32, tag="gb")
            nc.sync.dma_start(out=gb_sb[:],
                              in_=dmkap(pk32_in[:], GB_OFF, [[4, CPG], [1, 4]]))
            # zero all padded scratch volumes up front
            with tc.tile_pool(name="zpool", bufs=1) as zp:
                zero_sb = zp.tile([64, ZBLK], F32, tag="zsb")
                nc.vector.memset(zero_sb[:], 0.0)
                zero_dram(nc, zero_sb, xpad_d[:], 64, B * 10 * PLANE)
                zero_dram(nc, zero_sb, hpad_d[:], 64, B * 10 * PLANE)
                zero_dram(nc, zero_sb, xw_d[:], CPG, B * XVOL)
                zero_dram(nc, zero_sb, xw2_d[:], CPG, B * XVOL)

            # AllGather x slices (f16) -> xfull16, then cast passes
            nc.gpsimd.dma_start(
                out=xb16_d[:],
                in_=dmkap(pk16_in[:], XS_OFF, [[BV, CPG], [1, BV]]))
            nc.gpsimd.collective_compute(
                "AllGather", ALU.bypass, replica_groups=RG,
                ins=[xb16_d[:].opt()], outs=[xfull16_d[:].opt()])
            # xs cast to f32 (local slice; overlaps with the collective)
            with tc.tile_pool(name="xcast", bufs=2) as cp:
                for i in range(NCHK):
                    a16 = cp.tile([CPG, CSZ], F16, tag="a16")
                    nc.sync.dma_start(
                        out=a16[:],
                        in_=dmkap(pk16_in[:], XS_OFF + i * CSZ, [[BV, CPG], [1, CSZ]]))
                    a32 = cp.tile([CPG, CSZ], F32, tag="a32")
                    nc.vector.tensor_copy(out=a32[:], in_=a16[:])
                    nc.sync.dma_start(
                        out=dmkap(xs32_d[:], i * CSZ, [[BV, CPG], [1, CSZ]]),
                        in_=a32[:])
            tc.strict_bb_all_engine_barrier()
            # xfull cast to f32
            with tc.tile_pool(name="xfcast", bufs=2) as cp:
                for i in range(NCHK):
                    a16 = cp.tile([64, CSZ], F16, tag="b16")
                    nc.sync.dma_start(
                        out=a16[:],
                        in_=dmkap(xfull16_d[:], i * CSZ, [[BV, 64], [1, CSZ]]))
                    a32 = cp.tile([64, CSZ], F32, tag="b32")
                    nc.vector.tensor_copy(out=a32[:], in_=a16[:])
                    nc.sync.dma_start(
                        out=dmkap(xfull_d[:], i * CSZ, [[BV, 64], [1, CSZ]]),
                        in_=a32[:])
            tc.strict_bb_all_engine_barrier()
            dump("dxfull", xfull_d, 64, BV)

            # interior fills: xpad <- xfull, xw <- xs32
            for b in range(B):
                for d in range(D):
                    nc.sync.dma_start(
                        out=dmkap(xpad_d[:], (b * 10 + d + 1) * PLANE + 59,
                                  [[B * 10 * PLANE, 64], [58, 56], [1, 56]]),
                        in_=dmkap(xfull_d[:], b * V + d * 3136,
                                  [[BV, 64], [56, 56], [1, 56]]))
                    nc.sync.dma_start(
                        out=dmkap(xw_d[:], b * XVOL + (d + 3) * 3844 + 3 * 62 + 3,
                                  [[B * XVOL, CPG], [62, 56], [1, 56]]),
                        in_=dmkap(xs32_d[:], b * V + d * 3136,
                                  [[BV, CPG], [56, 56], [1, 56]]))
            tc.strict_bb_all_engine_barrier()

            dump("dxpad", xpad_d, 64, B * 10 * PLANE)
            dump("dxw", xw_d, CPG, B * XVOL)
            tc.strict_bb_all_engine_barrier()
            # ---- layer 1
            conv_phase(nc, tc, "1", xpad_d, wt1_sb, bo1_sb, off_d)
            tc.strict_bb_all_engine_barrier()
            dump("doff1", off_d, OCG, BV)
            dense_phase(nc, tc, "1", xw_d, off_d, wd1_sb, part_d, colsd_d)
            tc.strict_bb_all_engine_barrier()
            dump("dpart1", part_d, 64, BV)
            nc.gpsimd.collective_compute(
                "ReduceScatter", ALU.add, replica_groups=RG,
                ins=[part_d[:].opt()], outs=[h1s_d[:].opt()])
            tc.strict_bb_all_engine_barrier()
            dump("dh1s", h1s_d, CPG, BV)

            # ---- BN1 + relu -> hg1 (canonical), then AllGather -> hfull
            with tc.tile_pool(name="bn1p", bufs=1) as pool:
                scale, shift = bn_stats8(nc, tc, pool, "a", h1s_d[:], gb_sb[:, 0:1],
                                         gb_sb[:, 1:2])
                with tc.tile_pool(name="bn1ap", bufs=2) as apool:
                    for i in range(NCHK):
                        ht = apool.tile([CPG, CSZ], F32, tag="bnh1")
                        nc.sync.dma_start(
                            out=ht[:], in_=dmkap(h1s_d[:], i * CSZ, [[BV, CPG], [1, CSZ]]))
                        nc.scalar.activation(out=ht[:], in_=ht[:], func=ACTF.Relu,
                                             bias=shift[:], scale=scale[:])
                        deband_store(nc, ht, hg1_d[:], i)
            tc.strict_bb_all_engine_barrier()
            dump("dhg1", hg1_d, CPG, BV)
            nc.gpsimd.collective_compute(
                "AllGather", ALU.bypass, replica_groups=RG,
                ins=[hg1_d[:].opt()], outs=[hfull_d[:].opt()])
            tc.strict_bb_all_engine_barrier()
            dump("dhfull", hfull_d, 64, BV)

            # interior fills: hpad <- hfull, xw2 <- hg1
            for b in range(B):
                for d in range(D):
                    nc.sync.dma_start(
                        out=dmkap(hpad_d[:], (b * 10 + d + 1) * PLANE + 59,
                                  [[B * 10 * PLANE, 64], [58, 56], [1, 56]]),
                        in_=dmkap(hfull_d[:], b * V + d * 3136,
                                  [[BV, 64], [56, 56], [1, 56]]))
                    nc.sync.dma_start(
                        out=dmkap(xw2_d[:], b * XVOL + (d + 3) * 3844 + 3 * 62 + 3,
                                  [[B * XVOL, CPG], [62, 56], [1, 56]]),
                        in_=dmkap(hg1_d[:], b * V + d * 3136,
                                  [[BV, CPG], [56, 56], [1, 56]]))
            tc.strict_bb_all_engine_barrier()

            # ---- layer 2
            conv_phase(nc, tc, "2", hpad_d, wt2_sb, bo2_sb, off_d)
            tc.strict_bb_all_engine_barrier()
            dump("doff2", off_d, OCG, BV)
            dense_phase(nc, tc, "2", xw2_d, off_d, wd2_sb, part_d, colsd_d)
            tc.strict_bb_all_engine_barrier()
            dump("dpart2", part_d, 64, BV)
            nc.gpsimd.collective_compute(
                "ReduceScatter", ALU.add, replica_groups=RG,
                ins=[part_d[:].opt()], outs=[h2s_d[:].opt()])
            tc.strict_bb_all_engine_barrier()
            dump("dh2s", h2s_d, CPG, BV)

            # ---- BN2 + residual + relu -> out (canonical)
            with tc.tile_pool(name="bn2p", bufs=1) as pool:
                scale, shift = bn_stats8(nc, tc, pool, "b", h2s_d[:], gb_sb[:, 2:3],
                                         gb_sb[:, 3:4])
                with tc.tile_pool(name="bn2ap", bufs=1) as apool:
                    for i in range(NCHK):
                        ht = apool.tile([CPG, CSZ], F32, tag="bnh2")
                        rt = apool.tile([CPG, CSZ], F32, tag="bnr2")
                        nc.sync.dma_start(
                            out=ht[:], in_=dmkap(h2s_d[:], i * CSZ, [[BV, CPG], [1, CSZ]]))
                        band_load(nc, rt, xs32_d[:], i)
                        nc.vector.tensor_tensor(out=ht[:], in0=ht[:],
                                                in1=mkap(scale, 0, [[0, CSZ]]), op=ALU.mult)
                        nc.vector.tensor_tensor(out=ht[:], in0=ht[:],
                                                in1=mkap(shift, 0, [[0, CSZ]]), op=ALU.add)
                        nc.vector.tensor_tensor(out=ht[:], in0=ht[:], in1=rt[:], op=ALU.add)
                        ht16 = apool.tile([CPG, CSZ], F16, tag="bnh2c")
                        nc.vector.tensor_scalar(out=ht16[:], in0=ht[:], scalar1=0.0,
                                                scalar2=0.0, op0=ALU.max, op1=ALU.add)
                        deband_store(nc, ht16, out_d[:], i)
    return nc


# ---------------------------------------------------------- cached runner --
class Runner:
    """jit(shard_map(bass_exec)) built once; later calls only move data."""

    def __init__(self, nc, n_cores=NCORES):
        _b2j.install_neuronx_cc_hook()
        self.n_cores = n_cores
        partition_name = nc.partition_id_tensor.name if nc.partition_id_tensor else None
        in_names, out_names, out_avals, zero_shapes = [], [], [], []
        for alloc in nc.m.functions[0].allocations:
            if not isinstance(alloc, mybir.MemoryLocationSet):
                continue
            name = alloc.memorylocations[0].name
            if alloc.kind == "ExternalInput":
                if name != partition_name:
                    in_names.append(name)
            elif alloc.kind == "ExternalOutput":
                shape = tuple(alloc.tensor_shape)
                dtype = mybir.dt.np(alloc.dtype)
                out_names.append(name)
                out_avals.append(jax.core.ShapedArray(shape, dtype))
                zero_shapes.append((shape, dtype))
        self.n_params = len(in_names)
        self.in_names = list(in_names)
        self.out_names = out_names
        self.out_avals = out_avals
        all_in_names = list(in_names)
        if partition_name is not None:
            all_in_names.append(partition_name)

        def _body(*args):
            operands = list(args)
            if partition_name is not None:
                operands.append(_b2j.partition_id_tensor())
            outs = _b2j._bass_exec_p.bind(
                *operands,
                out_avals=tuple(out_avals),
                in_names=tuple(all_in_names),
                out_names=tuple(out_names),
                lowering_input_output_aliases=(),
                sim_require_finite=True,
                sim_require_nnan=True,
                nc=nc,
            )
            return tuple(outs)

        devices = jax.devices()[:n_cores]
        assert len(devices) == n_cores
        self.mesh = Mesh(np.asarray(devices), ("core",))
        self.fn = jax.jit(
            shard_map(_body, mesh=self.mesh,
                      in_specs=(PartitionSpec("core"),) * self.n_params,
                      out_specs=(PartitionSpec("core"),) * len(out_names),
                      check_rep=False),
            keep_unused=True,
        )

    def __call__(self, in_maps):
        concat_in = [
            np.concatenate([np.asarray(m[name]) for m in in_maps], axis=0)
            for name in self.in_names
        ]
        out_arrs = self.fn(*concat_in)
        outs = [np.asarray(a) for a in out_arrs]
        return [
            {
                name: outs[i].reshape(self.n_cores, *self.out_avals[i].shape)[c]
                for i, name in enumerate(self.out_names)
            }
            for c in range(self.n_cores)
        ]


_RUNNER = None


def _get_runner():
    global _RUNNER
    if _RUNNER is None:
        _RUNNER = Runner(build_fused())
    return _RUNNER


# ----------------------------------------------------------------- kernel --
def make_inmaps(inputs):
    x = np.ascontiguousarray(inputs["x"], dtype=np.float32)
    xt = np.ascontiguousarray(x.transpose(1, 0, 2, 3, 4)).reshape(64, BV)

    def wslices(w_off, b_off, w_dc):
        wts, bs, wds = [], [], []
        w_off = np.asarray(w_off, np.float32).reshape(G * OCG, 64, K)
        w_dc = np.asarray(w_dc, np.float32).reshape(64, G, CPG, K)
        b_off = np.asarray(b_off, np.float32)
        for g in range(G):
            wts.append(np.ascontiguousarray(
                w_off[g * OCG:(g + 1) * OCG].transpose(1, 2, 0)).reshape(64, -1))
            bs.append(np.ascontiguousarray(b_off[g * OCG:(g + 1) * OCG]).reshape(OCG, 1))
            wds.append(np.ascontiguousarray(
                w_dc[:, g].transpose(1, 2, 0)).reshape(CPG, -1))
        return wts, bs, wds

    wt1, bo1, wd1 = wslices(inputs["w_off1"], inputs["b_off1"], inputs["w_dc1"])
    wt2, bo2, wd2 = wslices(inputs["w_off2"], inputs["b_off2"], inputs["w_dc2"])
    g1 = np.asarray(inputs["gamma1"], np.float32)
    b1 = np.asarray(inputs["beta1"], np.float32)
    g2 = np.asarray(inputs["gamma2"], np.float32)
    b2 = np.asarray(inputs["beta2"], np.float32)

    in_maps = []
    for g in range(G):
        sl = slice(g * CPG, (g + 1) * CPG)
        gb = np.stack([g1[sl], b1[sl], g2[sl], b2[sl]], axis=1).astype(np.float32)
        pk16 = np.empty((1, N16), np.float16)
        pk16[0, XS_OFF:XS_OFF + CPG * BV] = xt[sl].reshape(-1)
        pk16[0, WT1_OFF:WT1_OFF + 64 * K * OCG] = wt1[g].reshape(-1)
        pk16[0, WT2_OFF:WT2_OFF + 64 * K * OCG] = wt2[g].reshape(-1)
        pk32 = np.empty((1, N32), np.float32)
        pk32[0, BO1_OFF:BO1_OFF + OCG] = bo1[g].reshape(-1)
        pk32[0, BO2_OFF:BO2_OFF + OCG] = bo2[g].reshape(-1)
        pk32[0, WD1_OFF:WD1_OFF + CPG * K * 64] = wd1[g].reshape(-1)
        pk32[0, WD2_OFF:WD2_OFF + CPG * K * 64] = wd2[g].reshape(-1)
        pk32[0, GB_OFF:GB_OFF + CPG * 4] = gb.reshape(-1)
        in_maps.append({"pk16": pk16, "pk32": pk32})
    return in_maps


def kernel(**inputs):
    runner = _get_runner()
    in_maps = make_inmaps(inputs)
    res = runner(in_maps)
    out = np.concatenate([r["out"] for r in res], axis=0)  # [64, BV] f16
    return np.ascontiguousarray(
        out.reshape(64, B, D, H, W).transpose(1, 0, 2, 3, 4)).astype(np.float32)


# revision 11
# speedup vs baseline: 1.0526x; 1.0526x over previous
"""Trainium2 Bass kernel for nn_DeformBasicBlock1 (deformable conv block).

Fully fused single-invocation SPMD program over 8 cores, group-sharded:
core g owns channel-group g (8 x-channels / 81 offset channels / 8 output
channels).  Cross-core exchange happens on device:
  AllGather(x slices) -> conv1 -> deform1 -> ReduceScatter(partials)
  -> per-channel BN1+relu -> AllGather -> conv2 -> deform2
  -> ReduceScatter -> BN2 + residual + relu -> per-core output slice.
The jitted executable is built once and cached in module globals; repeat
kernel() calls only move input/output slices (~35 MB) over the wire.
"""
import json
import numpy as np

import jax
import jax.numpy as jnp
from jax.sharding import Mesh, PartitionSpec, NamedSharding
from jax.experimental.shard_map import shard_map

import concourse.bass as bass
import concourse.mybir as mybir
from concourse.tile import TileContext
import concourse.bass_utils as bass_utils
import concourse.tile_utils as tile_utils

# ---------------------------------------------------------------- tilefix --
_orig_compile_bir_kernel = bass_utils.compile_bir_kernel


def _split_waits_json(bir_json: bytes) -> bytes:
    j = json.loads(bir_json)
    ctr = 0
    changed = False
    for f in j["functions"]:
        for b in f["blocks"]:
            insts = b["instructions"]
            if not any(
                len((i.get("sync_info") or {}).get("on_wait") or []) > 1
                for i in insts
            ):
                continue
            changed = True
            out = []
            for inst in insts:
                si = inst.get("sync_info")
                if si:
                    ow = si.get("on_wait") or []
                    if len(ow) > 1:
                        for w in ow[:-1]:
                            ctr += 1
                            nop = {
                                "engine": inst["engine"],
                                "ins": [],
                                "outs": [],
                                "name": f"WSPLIT-{ctr}",
                                "opcode": "NoOp",
                                "sync_info": {"on_update": [], "on_wait": [w]},
                            }
                            if "debug" in inst:
                                nop["debug"] = inst["debug"]
                            out.append(nop)
                        si["on_wait"] = [ow[-1]]
                out.append(inst)
            b["instructions"] = out
    return json.dumps(j).encode() if changed else bir_json


def _patched_compile_bir_kernel(bir_json, tmpdir, neff_name="file.neff"):
    if isinstance(bir_json, str):
        bir_json = bir_json.encode()
    return _orig_compile_bir_kernel(_split_waits_json(bir_json), tmpdir, neff_name)


bass_utils.compile_bir_kernel = _patched_compile_bir_kernel
import concourse.bass2jax as _b2j  # noqa: E402

_b2j.compile_bir_kernel = _patched_compile_bir_kernel
try:
    tile_utils.max_sbuf_usage = 204 * 1024
except Exception:
    pass

# ------------------------------------------------------------- constants --
B, D, H, W = 2, 8, 56, 56
CPG, G, K = 8, 8, 27
OCG = 81
V = D * H * W
BV = B * V
PLANE = 3364  # 58*58
NB, BH = 14, 4
P = NB * CPG  # 112
CH = D * BH * W  # 1792
XD, XH, XWW = 14, 10, 62
XSZ = XD * XH * XWW
XVOL = XD * 62 * 62
SS = 5
CLAMP = 1.999
F32 = mybir.dt.float32
F16 = mybir.dt.float16
AX = mybir.AxisListType
ALU = mybir.AluOpType
ACTF = mybir.ActivationFunctionType
NCORES = 8
RG = [list(range(NCORES))]

# packed-input blob layout (per core)
XS_OFF = 0
WT1_OFF = CPG * BV            # 401408
WT2_OFF = WT1_OFF + 64 * K * OCG  # + 139968
N16 = WT2_OFF + 64 * K * OCG  # 681344 f16 elements
BO1_OFF = 0
BO2_OFF = OCG
WD1_OFF = 2 * OCG
WD2_OFF = WD1_OFF + CPG * K * 64
GB_OFF = WD2_OFF + CPG * K * 64
N32 = GB_OFF + CPG * 4


def mkap(tile, off, dims):
    ap = tile[:]
    return bass.AP(tensor=ap.tensor, offset=ap.offset + off,
                   ap=[list(ap.ap[0])] + [list(d) for d in dims])


def dmkap(t_ap, off, dims):
    return bass.AP(tensor=t_ap.tensor, offset=t_ap.offset + off,
                   ap=[list(d) for d in dims])


ZBLK = 8192


def zero_dram(nc, zero_sb, dram_ap, rows, total):
    # stride-0 repeat DMAs corrupt data on this DMA engine; use one DMA per
    # block from a real zero tile instead.
    nblk = total // ZBLK
    rem = total - nblk * ZBLK
    for i in range(nblk):
        nc.sync.dma_start(out=dmkap(dram_ap, i * ZBLK, [[total, rows], [1, ZBLK]]),
                          in_=dmkap(zero_sb[:], 0, [[ZBLK, rows], [1, ZBLK]]))
    if rem:
        nc.sync.dma_start(out=dmkap(dram_ap, nblk * ZBLK, [[total, rows], [1, rem]]),
                          in_=dmkap(zero_sb[:], 0, [[ZBLK, rows], [1, rem]]))


def conv_phase(nc, tc, sfx, xpad_dram, wt_sb, bias_sb, off_dram):
    """27-tap conv: xpad_dram [64, B*10*PLANE] -> off_dram [81, B*D*3136]."""
    GUARD = 64
    CHUNKS = []
    for r0 in range(0, 58, 8):
        nr = min(8, 58 - r0)
        v0 = max(1, r0)
        v1 = min(57, r0 + nr)
        CHUNKS.append((r0 * 58, nr * 58, v0 - r0, v1 - v0))
    with tc.tile_pool(name=f"convp{sfx}", bufs=2) as pool, \
         tc.tile_pool(name=f"convps{sfx}", bufs=4, space="PSUM") as pspool:
        for b in range(B):
            for j in range(4):
                xpc = pool.tile([64, 2 * GUARD + 4 * PLANE], F32, tag="xpc")
                nc.vector.memset(xpc[:, :GUARD], 0.0)
                nc.vector.memset(xpc[:, GUARD + 4 * PLANE:], 0.0)
                nc.sync.dma_start(
                    out=xpc[:, GUARD:GUARD + 4 * PLANE],
                    in_=dmkap(xpad_dram[:], (b * 10 + 2 * j) * PLANE,
                              [[B * 10 * PLANE, 64], [1, 4 * PLANE]]))
                for ds in range(2):
                    d = 2 * j + ds
                    for (n0, nsz, vr, nv) in CHUNKS:
                        ps = pspool.tile([OCG, 512], F32, tag="cps")
                        for k in range(K):
                            kd, kh, kw = k // 9, (k // 3) % 3, k % 3
                            roff = GUARD + (ds + kd) * PLANE + (kh - 1) * 58 + (kw - 1) + n0
                            nc.tensor.matmul(ps[:, :nsz], wt_sb[:, k, :],
                                             mkap(xpc, roff, [[1, nsz]]),
                                             start=(k == 0), stop=(k == K - 1))
                        ot = pool.tile([OCG, 512], F32, tag="convot")
                        nc.vector.tensor_tensor(
                            out=ot[:, :nsz], in0=ps[:, :nsz],
                            in1=mkap(bias_sb, 0, [[0, nsz]]), op=ALU.add)
                        if nv <= 0:
                            continue
                        real_r0 = n0 // 58 + vr - 1
                        nc.sync.dma_start(
                            out=dmkap(off_dram[:], (b * D + d) * 3136 + real_r0 * 56,
                                      [[B * D * 3136, OCG], [1, nv * 56]]),
                            in_=mkap(ot, vr * 58 + 1, [[58, nv], [1, 56]]))


def dense_phase(nc, tc, sfx, xw_dram, off_dram, wd_sb, partial_dram, colsd_dram):
    """Dense 5^3 deform + einsum -> partial_dram [64, BV] (band-perm)."""
    with tc.tile_pool(name=f"densep{sfx}", bufs=1) as pool, \
         tc.tile_pool(name=f"densew{sfx}", bufs=1) as wpool, \
         tc.tile_pool(name=f"denseps{sfx}", bufs=2, space="PSUM") as pspool:
        for b in range(B):
            xw = pool.tile([P, XSZ], F32, tag="xw")
            for dd in range(XD):
                nc.sync.dma_start(
                    out=mkap(xw, dd * XH * XWW, [[1, 620]]),
                    in_=dmkap(xw_dram[:], b * XVOL + dd * 62 * 62,
                              [[BH * XWW, NB], [B * XVOL, CPG], [1, XH * XWW]]))
            for k in range(K):
                kd, kh, kw = k // 9 - 1, (k // 3) % 3 - 1, k % 3 - 1
                offt = pool.tile([P, 3, CH], F32, tag="offt")
                for ax in range(3):
                    for dd in range(D):
                        nc.sync.dma_start(
                            out=mkap(offt, ax * CH + dd * BH * W, [[1, BH * W]]),
                            in_=dmkap(off_dram[:],
                                      (3 * k + ax) * B * D * 3136 + (b * D + dd) * 3136,
                                      [[BH * W, NB], [0, CPG], [1, BH * W]]))
                nc.vector.tensor_scalar(out=offt[:], in0=offt[:], scalar1=CLAMP,
                                        scalar2=-CLAMP, op0=ALU.min, op1=ALU.max)
                hw = pool.tile([P, SS, CH], F32, tag="hw")
                for a in range(SS):
                    nc.scalar.activation(hw[:, a, :], offt[:, 2, :], ACTF.Abs,
                                         bias=float(-(a - 2)), scale=1.0)
                    nc.scalar.activation(hw[:, a, :], hw[:, a, :], ACTF.Relu,
                                         bias=1.0, scale=-1.0)
                cols = wpool.tile([P, CH], F32, tag="cols")
                pt = wpool.tile([P, CH], F32, tag="pt")
                at = wpool.tile([P, CH], F32, tag="at")
                tt = wpool.tile([P, CH], F32, tag="tt")
                hdsl = pool.tile([P, CH], F32, tag="hdsl")
                hhsl = pool.tile([P, CH], F32, tag="hhsl")
                first = True
                for sd in range(SS):
                    nc.scalar.activation(hdsl[:], offt[:, 0, :], ACTF.Abs,
                                         bias=float(-(sd - 2)), scale=1.0)
                    nc.scalar.activation(hdsl[:], hdsl[:], ACTF.Relu,
                                         bias=1.0, scale=-1.0)
                    for sh in range(SS):
                        nc.scalar.activation(hhsl[:], offt[:, 1, :], ACTF.Abs,
                                             bias=float(-(sh - 2)), scale=1.0)
                        nc.scalar.activation(hhsl[:], hhsl[:], ACTF.Relu,
                                             bias=1.0, scale=-1.0)
                        nc.vector.tensor_tensor(out=pt[:], in0=hdsl[:],
                                                in1=hhsl[:], op=ALU.mult)
                        for sw in range(SS):
                            xoff = ((1 + kd + sd) * XH * XWW + (1 + kh + sh) * XWW
                                    + (1 + kw + sw))
                            xap = mkap(xw, xoff, [[XH * XWW, D], [XWW, BH], [1, W]])
                            dst = at if sw == 0 else tt
                            nc.vector.tensor_tensor(out=dst[:], in0=xap,
                                                    in1=hw[:, sw, :], op=ALU.mult)
                            if sw > 0:
                                nc.vector.tensor_tensor(out=at[:], in0=at[:],
                                                        in1=tt[:], op=ALU.add)
                        if first:
                            nc.vector.tensor_tensor(out=cols[:], in0=pt[:], in1=at[:],
                                                    op=ALU.mult)
                            first = False
                        else:
                            nc.gpsimd.tensor_tensor(out=tt[:], in0=pt[:], in1=at[:],
                                                    op=ALU.mult)
                            nc.gpsimd.tensor_tensor(out=cols[:], in0=cols[:], in1=tt[:],
                                                    op=ALU.add)
                nc.sync.dma_start(
                    out=dmkap(colsd_dram[:], (b * K + k) * CH,
                              [[B * K * CH, P], [1, CH]]),
                    in_=cols[:])
            tc.strict_bb_all_engine_barrier()
            for hb in range(NB):
                ps2 = pspool.tile([64, 2048], F32, tag="eps")
                for k in range(K):
                    cr = wpool.tile([CPG, CH], F32, tag="colsr")
                    nc.sync.dma_start(
                        out=cr[:],
                        in_=dmkap(colsd_dram[:], hb * CPG * B * K * CH + (b * K + k) * CH,
                                  [[B * K * CH, CPG], [1, CH]]))
                    for i in range(4):
                        nc.tensor.matmul(ps2[:, i * 512:i * 512 + 448], wd_sb[:, k, :],
                                         cr[:, i * 448:(i + 1) * 448],
                                         start=(k == 0), stop=(k == K - 1))
                pot = wpool.tile([64, CH], F32, tag="pot")
                nc.vector.tensor_copy(out=pot[:], in_=mkap(ps2, 0, [[512, 4], [1, 448]]))
                nc.sync.dma_start(
                    out=dmkap(partial_dram[:], b * V + hb * CH, [[BV, 64], [1, CH]]),
                    in_=pot[:])


def ensure_consts(nc):
    for v in (2.0, -2.0, -1.0, 1e-5):
        key = (F32, v)
        if key not in nc.const_aps.aps:
            t = nc.alloc_sbuf_tensor(f"const-f32-{v}", [128, 1], F32)
            nc.gpsimd.memset(t.ap(), v)
            nc.const_aps.aps[key] = t.ap()


def bn_stats8(nc, tc, pool, sfx, src_dram, gamma_sb, beta_sb):
    """BN scale/shift for [CPG, BV] shard (band layout is irrelevant)."""
    sum_t = pool.tile([CPG, 1], F32, tag=f"bnsum{sfx}")
    sq_t = pool.tile([CPG, 1], F32, tag=f"bnsq{sfx}")
    t1 = pool.tile([CPG, 1], F32, tag=f"bnt1{sfx}")
    t2 = pool.tile([CPG, 1], F32, tag=f"bnt2{sfx}")
    with tc.tile_pool(name=f"bnstat{sfx}", bufs=1) as big:
        for i in range(NCHK):
            ht = big.tile([CPG, CSZ], F32, tag=f"bnh{sfx}")
            sqv = big.tile([CPG, CSZ], F32, tag=f"bnsqv{sfx}")
            nc.sync.dma_start(out=ht[:], in_=dmkap(src_dram, i * CSZ, [[BV, CPG], [1, CSZ]]))
            nc.vector.tensor_reduce(out=t1[:], in_=ht[:], axis=AX.X, op=ALU.add)
            nc.vector.tensor_tensor(out=sqv[:], in0=ht[:], in1=ht[:], op=ALU.mult)
            nc.vector.tensor_reduce(out=t2[:], in_=sqv[:], axis=AX.X, op=ALU.add)
            if i == 0:
                nc.vector.tensor_copy(out=sum_t[:], in_=t1[:])
                nc.vector.tensor_copy(out=sq_t[:], in_=t2[:])
            else:
                nc.vector.tensor_tensor(out=sum_t[:], in0=sum_t[:], in1=t1[:], op=ALU.add)
                nc.vector.tensor_tensor(out=sq_t[:], in0=sq_t[:], in1=t2[:], op=ALU.add)
    N = float(BV)
    scale = pool.tile([CPG, 1], F32, tag=f"bnscale{sfx}")
    shift = pool.tile([CPG, 1], F32, tag=f"bnshift{sfx}")
    mean = t1
    nc.vector.tensor_scalar(out=mean[:], in0=sum_t[:], scalar1=1.0 / N, scalar2=0.0,
                            op0=ALU.mult, op1=ALU.add)
    var = t2
    nc.vector.tensor_scalar(out=var[:], in0=sq_t[:], scalar1=1.0 / N, scalar2=0.0,
                            op0=ALU.mult, op1=ALU.add)
    msq = pool.tile([CPG, 1], F32, tag=f"bnmsq{sfx}")
    nc.vector.tensor_tensor(out=msq[:], in0=mean[:], in1=mean[:], op=ALU.mult)
    nc.vector.tensor_tensor(out=var[:], in0=var[:], in1=msq[:], op=ALU.subtract)
    rstd = pool.tile([CPG, 1], F32, tag=f"bnrstd{sfx}")
    nc.scalar.activation(out=rstd[:], in_=var[:], func=ACTF.Sqrt, bias=1e-5, scale=1.0)
    nc.vector.reciprocal(out=rstd[:], in_=rstd[:])
    nc.vector.tensor_tensor(out=scale[:], in0=gamma_sb[:], in1=rstd[:], op=ALU.mult)
    nc.vector.tensor_tensor(out=shift[:], in0=mean[:], in1=scale[:], op=ALU.mult)
    nc.vector.tensor_tensor(out=shift[:], in0=beta_sb[:], in1=shift[:], op=ALU.subtract)
    return scale, shift


NCHK = 4
CSZ = BV // NCHK


def deband_store(nc, src_tile, dst_dram, i):
    """Store band-layout chunk i of [CPG, CSZ] to canonical [CPG, BV] DRAM."""
    b, half = i // 2, i % 2
    for hbr in range(7):
        hb = half * 7 + hbr
        nc.sync.dma_start(
            out=dmkap(dst_dram, b * V + hb * BH * W, [[BV, CPG], [3136, D], [1, BH * W]]),
            in_=mkap(src_tile, hbr * CH, [[BH * W, D], [1, BH * W]]))


def band_load(nc, dst_tile, src_dram, i):
    """Load canonical [CPG, BV] DRAM into band-layout chunk i [CPG, CSZ]."""
    b, half = i // 2, i % 2
    for hbr in range(7):
        hb = half * 7 + hbr
        nc.sync.dma_start(
            out=mkap(dst_tile, hbr * CH, [[BH * W, D], [1, BH * W]]),
            in_=dmkap(src_dram, b * V + hb * BH * W, [[BV, CPG], [3136, D], [1, BH * W]]))


# ------------------------------------------------------------ program nc --
def build_fused(debug=False):
    nc = bass.Bass("TRN2", target_bir_lowering=False, num_devices=NCORES)
    ensure_consts(nc)
    pk16_in = nc.declare_dram_parameter("pk16", [1, N16], F16, isOutput=False)
    pk32_in = nc.declare_dram_parameter("pk32", [1, N32], F32, isOutput=False)
    out_d = nc.declare_dram_parameter("out", [64, BV], F16, isOutput=True)
    og16_d = nc.dram_tensor("og16", [CPG, BV], F16)
    ofull16_d = nc.dram_tensor("ofull16", [64, BV], F16, addr_space="Shared")

    xs32_d = nc.dram_tensor("xs32", [CPG, BV], F32)
    xb16_d = nc.dram_tensor("xb16", [CPG, BV], F16)
    xfull16_d = nc.dram_tensor("xfull16", [64, BV], F16, addr_space="Shared")
    xfull_d = nc.dram_tensor("xfull", [64, BV], F32)
    xpad_d = nc.dram_tensor("xpad", [64, B * 10 * PLANE], F32)
    xw_d = nc.dram_tensor("xw", [CPG, B * XVOL], F32)
    xw2_d = nc.dram_tensor("xw2", [CPG, B * XVOL], F32)
    hpad_d = nc.dram_tensor("hpad", [64, B * 10 * PLANE], F32)
    off_d = nc.dram_tensor("offs", [OCG, B * D * 3136], F32)
    colsd_d = nc.dram_tensor("colsd", [P, B * K * CH], F32)
    part_d = nc.dram_tensor("part", [64, BV], F32)
    h1s_d = nc.dram_tensor("h1s", [CPG, BV], F32)
    h2s_d = nc.dram_tensor("h2s", [CPG, BV], F32)
    hg1_d = nc.dram_tensor("hg1", [CPG, BV], F32)
    hfull_d = nc.dram_tensor("hfull", [64, BV], F32, addr_space="Shared")

    dbg = {}
    if debug:
        for nm, shp in (("dxfull", [64, BV]), ("doff1", [OCG, BV]),
                        ("dpart1", [64, BV]), ("dh1s", [CPG, BV]),
                        ("dhg1", [CPG, BV]), ("dhfull", [64, BV]),
                        ("doff2", [OCG, BV]), ("dpart2", [64, BV]),
                        ("dh2s", [CPG, BV]), ("dxpad", [64, B * 10 * PLANE]),
                        ("dxw", [CPG, B * XVOL])):
            dbg[nm] = nc.declare_dram_parameter(nm, shp, F32, isOutput=True)

    def dump(nm, src, rows, total):
        if not debug:
            return
        nc.sync.dma_start(out=dmkap(dbg[nm][:], 0, [[total, rows], [1, total]]),
                          in_=dmkap(src[:], 0, [[total, rows], [1, total]]))

    with TileContext(nc) as tc:
        with tc.tile_pool(name="single", bufs=1) as sp:
            wt1_sb = sp.tile([64, K, OCG], F32, tag="wt1")
            wt2_sb = sp.tile([64, K, OCG], F32, tag="wt2")
            for woff, wsb, wtag in ((WT1_OFF, wt1_sb, "w16a"), (WT2_OFF, wt2_sb, "w16b")):
                w16 = sp.tile([64, K * OCG], F16, tag=wtag)
                nc.sync.dma_start(
                    out=w16[:],
                    in_=dmkap(pk16_in[:], woff, [[K * OCG, 64], [1, K * OCG]]))
                nc.vector.tensor_copy(out=wsb[:], in_=w16[:])
            bo1_sb = sp.tile([OCG, 1], F32, tag="bo1")
            nc.sync.dma_start(out=bo1_sb[:],
                              in_=dmkap(pk32_in[:], BO1_OFF, [[1, OCG], [1, 1]]))
            bo2_sb = sp.tile([OCG, 1], F32, tag="bo2")
            nc.sync.dma_start(out=bo2_sb[:],
                              in_=dmkap(pk32_in[:], BO2_OFF, [[1, OCG], [1, 1]]))
            wd1_sb = sp.tile([CPG, K, 64], F32, tag="wd1")
            nc.sync.dma_start(out=wd1_sb[:],
                              in_=dmkap(pk32_in[:], WD1_OFF, [[K * 64, CPG], [1, K * 64]]))
            wd2_sb = sp.tile([CPG, K, 64], F32, tag="wd2")
            nc.sync.dma_start(out=wd2_sb[:],
                              in_=dmkap(pk32_in[:], WD2_OFF, [[K * 64, CPG], [1, K * 64]]))
            gb_sb = sp.tile([CPG, 4], F32, tag="gb")
            nc.sync.dma_start(out=gb_sb[:],
                              in_=dmkap(pk32_in[:], GB_OFF, [[4, CPG], [1, 4]]))
            # zero all padded scratch volumes up front
            with tc.tile_pool(name="zpool", bufs=1) as zp:
                zero_sb = zp.tile([64, ZBLK], F32, tag="zsb")
                nc.vector.memset(zero_sb[:], 0.0)
                zero_dram(nc, zero_sb, xpad_d[:], 64, B * 10 * PLANE)
                zero_dram(nc, zero_sb, hpad_d[:], 64, B * 10 * PLANE)
                zero_dram(nc, zero_sb, xw_d[:], CPG, B * XVOL)
                zero_dram(nc, zero_sb, xw2_d[:], CPG, B * XVOL)

            # AllGather x slices (f16) -> xfull16, then cast passes
            nc.gpsimd.dma_start(
                out=xb16_d[:],
                in_=dmkap(pk16_in[:], XS_OFF, [[BV, CPG], [1, BV]]))
            nc.gpsimd.collective_compute(
                "AllGather", ALU.bypass, replica_groups=RG,
                ins=[xb16_d[:].opt()], outs=[xfull16_d[:].opt()])
            # xs cast to f32 (local slice; overlaps with the collective)
            with tc.tile_pool(name="xcast", bufs=2) as cp:
                for i in range(NCHK):
                    a16 = cp.tile([CPG, CSZ], F16, tag="a16")
                    nc.sync.dma_start(
                        out=a16[:],
                        in_=dmkap(pk16_in[:], XS_OFF + i * CSZ, [[BV, CPG], [1, CSZ]]))
                    a32 = cp.tile([CPG, CSZ], F32, tag="a32")
                    nc.vector.tensor_copy(out=a32[:], in_=a16[:])
                    nc.sync.dma_start(
                        out=dmkap(xs32_d[:], i * CSZ, [[BV, CPG], [1, CSZ]]),
                        in_=a32[:])
            tc.strict_bb_all_engine_barrier()
            # xfull cast to f32
            with tc.tile_pool(name="xfcast", bufs=2) as cp:
                for i in range(NCHK):
                    a16 = cp.tile([64, CSZ], F16, tag="b16")
                    nc.sync.dma_start(
                        out=a16[:],
                        in_=dmkap(xfull16_d[:], i * CSZ, [[BV, 64], [1, CSZ]]))
                    a32 = cp.tile([64, CSZ], F32, tag="b32")
                    nc.vector.tensor_copy(out=a32[:], in_=a16[:])
                    nc.sync.dma_start(
                        out=dmkap(xfull_d[:], i * CSZ, [[BV, 64], [1, CSZ]]),
                        in_=a32[:])
            tc.strict_bb_all_engine_barrier()
            dump("dxfull", xfull_d, 64, BV)

            # interior fills: xpad <- xfull, xw <- xs32
            for b in range(B):
                for d in range(D):
                    nc.sync.dma_start(
                        out=dmkap(xpad_d[:], (b * 10 + d + 1) * PLANE + 59,
                                  [[B * 10 * PLANE, 64], [58, 56], [1, 56]]),
                        in_=dmkap(xfull_d[:], b * V + d * 3136,
                                  [[BV, 64], [56, 56], [1, 56]]))
                    nc.sync.dma_start(
                        out=dmkap(xw_d[:], b * XVOL + (d + 3) * 3844 + 3 * 62 + 3,
                                  [[B * XVOL, CPG], [62, 56], [1, 56]]),
                        in_=dmkap(xs32_d[:], b * V + d * 3136,
                                  [[BV, CPG], [56, 56], [1, 56]]))
            tc.strict_bb_all_engine_barrier()

            dump("dxpad", xpad_d, 64, B * 10 * PLANE)
            dump("dxw", xw_d, CPG, B * XVOL)
            tc.strict_bb_all_engine_barrier()
            # ---- layer 1
            conv_phase(nc, tc, "1", xpad_d, wt1_sb, bo1_sb, off_d)
            tc.strict_bb_all_engine_barrier()
            dump("doff1", off_d, OCG, BV)
            dense_phase(nc, tc, "1", xw_d, off_d, wd1_sb, part_d, colsd_d)
            tc.strict_bb_all_engine_barrier()
            dump("dpart1", part_d, 64, BV)
            nc.gpsimd.collective_compute(
                "ReduceScatter", ALU.add, replica_groups=RG,
                ins=[part_d[:].opt()], outs=[h1s_d[:].opt()])
            tc.strict_bb_all_engine_barrier()
            dump("dh1s", h1s_d, CPG, BV)

            # ---- BN1 + relu -> hg1 (canonical), then AllGather -> hfull
            with tc.tile_pool(name="bn1p", bufs=1) as pool:
                scale, shift = bn_stats8(nc, tc, pool, "a", h1s_d[:], gb_sb[:, 0:1],
                                         gb_sb[:, 1:2])
                with tc.tile_pool(name="bn1ap", bufs=2) as apool:
                    for i in range(NCHK):
                        ht = apool.tile([CPG, CSZ], F32, tag="bnh1")
                        nc.sync.dma_start(
                            out=ht[:], in_=dmkap(h1s_d[:], i * CSZ, [[BV, CPG], [1, CSZ]]))
                        nc.scalar.activation(out=ht[:], in_=ht[:], func=ACTF.Relu,
                                             bias=shift[:], scale=scale[:])
                        deband_store(nc, ht, hg1_d[:], i)
            tc.strict_bb_all_engine_barrier()
            dump("dhg1", hg1_d, CPG, BV)
            nc.gpsimd.collective_compute(
                "AllGather", ALU.bypass, replica_groups=RG,
                ins=[hg1_d[:].opt()], outs=[hfull_d[:].opt()])
            tc.strict_bb_all_engine_barrier()
            dump("dhfull", hfull_d, 64, BV)

            # interior fills: hpad <- hfull, xw2 <- hg1
            for b in range(B):
                for d in range(D):
                    nc.sync.dma_start(
                        out=dmkap(hpad_d[:], (b * 10 + d + 1) * PLANE + 59,
                                  [[B * 10 * PLANE, 64], [58, 56], [1, 56]]),
                        in_=dmkap(hfull_d[:], b * V + d * 3136,
                                  [[BV, 64], [56, 56], [1, 56]]))
                    nc.sync.dma_start(
                        out=dmkap(xw2_d[:], b * XVOL + (d + 3) * 3844 + 3 * 62 + 3,
                                  [[B * XVOL, CPG], [62, 56], [1, 56]]),
                        in_=dmkap(hg1_d[:], b * V + d * 3136,
                                  [[BV, CPG], [56, 56], [1, 56]]))
            tc.strict_bb_all_engine_barrier()

            # ---- layer 2
            conv_phase(nc, tc, "2", hpad_d, wt2_sb, bo2_sb, off_d)
            tc.strict_bb_all_engine_barrier()
            dump("doff2", off_d, OCG, BV)
            dense_phase(nc, tc, "2", xw2_d, off_d, wd2_sb, part_d, colsd_d)
            tc.strict_bb_all_engine_barrier()
            dump("dpart2", part_d, 64, BV)
            nc.gpsimd.collective_compute(
                "ReduceScatter", ALU.add, replica_groups=RG,
                ins=[part_d[:].opt()], outs=[h2s_d[:].opt()])
            tc.strict_bb_all_engine_barrier()
            dump("dh2s", h2s_d, CPG, BV)

            # ---- BN2 + residual + relu -> out (canonical)
            with tc.tile_pool(name="bn2p", bufs=1) as pool:
                scale, shift = bn_stats8(nc, tc, pool, "b", h2s_d[:], gb_sb[:, 2:3],
                                         gb_sb[:, 3:4])
                with tc.tile_pool(name="bn2ap", bufs=1) as apool:
                    for i in range(NCHK):
                        ht = apool.tile([CPG, CSZ], F32, tag="bnh2")
                        rt = apool.tile([CPG, CSZ], F32, tag="bnr2")
                        nc.sync.dma_start(
                            out=ht[:], in_=dmkap(h2s_d[:], i * CSZ, [[BV, CPG], [1, CSZ]]))
                        band_load(nc, rt, xs32_d[:], i)
                        nc.vector.tensor_tensor(out=ht[:], in0=ht[:],
                                                in1=mkap(scale, 0, [[0, CSZ]]), op=ALU.mult)
                        nc.vector.tensor_tensor(out=ht[:], in0=ht[:],
                                                in1=mkap(shift, 0, [[0, CSZ]]), op=ALU.add)
                        nc.vector.tensor_tensor(out=ht[:], in0=ht[:], in1=rt[:], op=ALU.add)
                        ht16 = apool.tile([CPG, CSZ], F16, tag="bnh2c")
                        nc.vector.tensor_scalar(out=ht16[:], in0=ht[:], scalar1=0.0,
                                                scalar2=0.0, op0=ALU.max, op1=ALU.add)
                        deband_store(nc, ht16, og16_d[:], i)
            # replicate the output on every core so the host fetches 1 shard
            tc.strict_bb_all_engine_barrier()
            nc.gpsimd.collective_compute(
                "AllGather", ALU.bypass, replica_groups=RG,
                ins=[og16_d[:].opt()], outs=[ofull16_d[:].opt()])
            tc.strict_bb_all_engine_barrier()
            nc.sync.dma_start(out=out_d[:], in_=ofull16_d[:])
    return nc


# ---------------------------------------------------------- cached runner --
class Runner:
    """jit(shard_map(bass_exec)) built once; later calls only move data."""

    def __init__(self, nc, n_cores=NCORES):
        _b2j.install_neuronx_cc_hook()
        self.n_cores = n_cores
        partition_name = nc.partition_id_tensor.name if nc.partition_id_tensor else None
        in_names, out_names, out_avals, zero_shapes = [], [], [], []
        for alloc in nc.m.functions[0].allocations:
            if not isinstance(alloc, mybir.MemoryLocationSet):
                continue
            name = alloc.memorylocations[0].name
            if alloc.kind == "ExternalInput":
                if name != partition_name:
                    in_names.append(name)
            elif alloc.kind == "ExternalOutput":
                shape = tuple(alloc.tensor_shape)
                dtype = mybir.dt.np(alloc.dtype)
                out_names.append(name)
                out_avals.append(jax.core.ShapedArray(shape, dtype))
                zero_shapes.append((shape, dtype))
        self.n_params = len(in_names)
        self.in_names = list(in_names)
        self.out_names = out_names
        self.out_avals = out_avals
        all_in_names = list(in_names)
        if partition_name is not None:
            all_in_names.append(partition_name)

        def _body(*args):
            operands = list(args)
            if partition_name is not None:
                operands.append(_b2j.partition_id_tensor())
            outs = _b2j._bass_exec_p.bind(
                *operands,
                out_avals=tuple(out_avals),
                in_names=tuple(all_in_names),
                out_names=tuple(out_names),
                lowering_input_output_aliases=(),
                sim_require_finite=True,
                sim_require_nnan=True,
                nc=nc,
            )
            return tuple(outs)

        devices = jax.devices()[:n_cores]
        assert len(devices) == n_cores
        self.mesh = Mesh(np.asarray(devices), ("core",))
        self.fn = jax.jit(
            shard_map(_body, mesh=self.mesh,
                      in_specs=(PartitionSpec("core"),) * self.n_params,
                      out_specs=(PartitionSpec("core"),) * len(out_names),
                      check_rep=False),
            keep_unused=True,
        )

    def __call__(self, in_maps):
        concat_in = [
            np.concatenate([np.asarray(m[name]) for m in in_maps], axis=0)
            for name in self.in_names
        ]
        out_arrs = self.fn(*concat_in)
        outs = [np.asarray(a) for a in out_arrs]
        return [
            {
                name: outs[i].reshape(self.n_cores, *self.out_avals[i].shape)[c]
                for i, name in enumerate(self.out_names)
            }
            for c in range(self.n_cores)
        ]


_RUNNER = None


def _get_runner():
    global _RUNNER
    if _RUNNER is None:
        _RUNNER = Runner(build_fused())
    return _RUNNER


# ----------------------------------------------------------------- kernel --
def make_inmaps(inputs):
    x = np.ascontiguousarray(inputs["x"], dtype=np.float32)
    xt = np.ascontiguousarray(x.transpose(1, 0, 2, 3, 4)).reshape(64, BV)

    def wslices(w_off, b_off, w_dc):
        wts, bs, wds = [], [], []
        w_off = np.asarray(w_off, np.float32).reshape(G * OCG, 64, K)
        w_dc = np.asarray(w_dc, np.float32).reshape(64, G, CPG, K)
        b_off = np.asarray(b_off, np.float32)
        for g in range(G):
            wts.append(np.ascontiguousarray(
                w_off[g * OCG:(g + 1) * OCG].transpose(1, 2, 0)).reshape(64, -1))
            bs.append(np.ascontiguousarray(b_off[g * OCG:(g + 1) * OCG]).reshape(OCG, 1))
            wds.append(np.ascontiguousarray(
                w_dc[:, g].transpose(1, 2, 0)).reshape(CPG, -1))
        return wts, bs, wds

    wt1, bo1, wd1 = wslices(inputs["w_off1"], inputs["b_off1"], inputs["w_dc1"])
    wt2, bo2, wd2 = wslices(inputs["w_off2"], inputs["b_off2"], inputs["w_dc2"])
    g1 = np.asarray(inputs["gamma1"], np.float32)
    b1 = np.asarray(inputs["beta1"], np.float32)
    g2 = np.asarray(inputs["gamma2"], np.float32)
    b2 = np.asarray(inputs["beta2"], np.float32)

    in_maps = []
    for g in range(G):
        sl = slice(g * CPG, (g + 1) * CPG)
        gb = np.stack([g1[sl], b1[sl], g2[sl], b2[sl]], axis=1).astype(np.float32)
        pk16 = np.empty((1, N16), np.float16)
        pk16[0, XS_OFF:XS_OFF + CPG * BV] = xt[sl].reshape(-1)
        pk16[0, WT1_OFF:WT1_OFF + 64 * K * OCG] = wt1[g].reshape(-1)
        pk16[0, WT2_OFF:WT2_OFF + 64 * K * OCG] = wt2[g].reshape(-1)
        pk32 = np.empty((1, N32), np.float32)
        pk32[0, BO1_OFF:BO1_OFF + OCG] = bo1[g].reshape(-1)
        pk32[0, BO2_OFF:BO2_OFF + OCG] = bo2[g].reshape(-1)
        pk32[0, WD1_OFF:WD1_OFF + CPG * K * 64] = wd1[g].reshape(-1)
        pk32[0, WD2_OFF:WD2_OFF + CPG * K * 64] = wd2[g].reshape(-1)
        pk32[0, GB_OFF:GB_OFF + CPG * 4] = gb.reshape(-1)
        in_maps.append({"pk16": pk16, "pk32": pk32})
    return in_maps


def kernel(**inputs):
    runner = _get_runner()
    in_maps = make_inmaps(inputs)
    concat_in = [
        np.concatenate([np.asarray(m[name]) for m in in_maps], axis=0)
        for name in runner.in_names
    ]
    out_arrs = runner.fn(*concat_in)
    oi = runner.out_names.index("out")
    # output is AllGather-replicated across cores; fetch a single shard
    out = np.asarray(out_arrs[oi].addressable_shards[0].data)  # [64, BV] f16
    return np.ascontiguousarray(
        out.reshape(64, B, D, H, W).transpose(1, 0, 2, 3, 4)).astype(np.float32)


# revision 17
# speedup vs baseline: 1.2048x; 1.1446x over previous
"""Trainium2 Bass kernel for nn_DeformBasicBlock1 (deformable conv block).

Fully fused single-invocation SPMD program over 8 cores, group-sharded:
core g owns channel-group g (8 x-channels / 81 offset channels / 8 output
channels).  Cross-core exchange happens on device:
  AllGather(x slices) -> conv1 -> deform1 -> ReduceScatter(partials)
  -> per-channel BN1+relu -> AllGather -> conv2 -> deform2
  -> ReduceScatter -> BN2 + residual + relu -> per-core output slice.
The jitted executable is built once and cached in module globals; repeat
kernel() calls only move input/output slices (~35 MB) over the wire.
"""
import json
import numpy as np

import jax
import jax.numpy as jnp
from jax.sharding import Mesh, PartitionSpec, NamedSharding
from jax.experimental.shard_map import shard_map

import concourse.bass as bass
import concourse.mybir as mybir
from concourse.tile import TileContext
import concourse.bass_utils as bass_utils
import concourse.tile_utils as tile_utils

# ---------------------------------------------------------------- tilefix --
_orig_compile_bir_kernel = bass_utils.compile_bir_kernel


def _split_waits_json(bir_json: bytes) -> bytes:
    j = json.loads(bir_json)
    ctr = 0
    changed = False
    for f in j["functions"]:
        for b in f["blocks"]:
            insts = b["instructions"]
            if not any(
                len((i.get("sync_info") or {}).get("on_wait") or []) > 1
                for i in insts
            ):
                continue
            changed = True
            out = []
            for inst in insts:
                si = inst.get("sync_info")
                if si:
                    ow = si.get("on_wait") or []
                    if len(ow) > 1:
                        for w in ow[:-1]:
                            ctr += 1
                            nop = {
                                "engine": inst["engine"],
                                "ins": [],
                                "outs": [],
                                "name": f"WSPLIT-{ctr}",
                                "opcode": "NoOp",
                                "sync_info": {"on_update": [], "on_wait": [w]},
                            }
                            if "debug" in inst:
                                nop["debug"] = inst["debug"]
                            out.append(nop)
                        si["on_wait"] = [ow[-1]]
                out.append(inst)
            b["instructions"] = out
    return json.dumps(j).encode() if changed else bir_json


def _patched_compile_bir_kernel(bir_json, tmpdir, neff_name="file.neff"):
    if isinstance(bir_json, str):
        bir_json = bir_json.encode()
    return _orig_compile_bir_kernel(_split_waits_json(bir_json), tmpdir, neff_name)


bass_utils.compile_bir_kernel = _patched_compile_bir_kernel
import concourse.bass2jax as _b2j  # noqa: E402

_b2j.compile_bir_kernel = _patched_compile_bir_kernel
try:
    tile_utils.max_sbuf_usage = 204 * 1024
except Exception:
    pass

# ------------------------------------------------------------- constants --
B, D, H, W = 2, 8, 56, 56
CPG, G, K = 8, 8, 27
OCG = 81
V = D * H * W
BV = B * V
PLANE = 3364  # 58*58
NB, BH = 14, 4
P = NB * CPG  # 112
CH = D * BH * W  # 1792
XD, XH, XWW = 14, 10, 62
XSZ = XD * XH * XWW
XVOL = XD * 62 * 62
SS = 5
CLAMP = 1.999
F32 = mybir.dt.float32
F16 = mybir.dt.float16
AX = mybir.AxisListType
ALU = mybir.AluOpType
ACTF = mybir.ActivationFunctionType
NCORES = 8
RG = [list(range(NCORES))]

# packed-input blob layout (per core)
XS_OFF = 0
NX16 = CPG * BV               # x blob: 401408 f16 elements
WT1_OFF = 0
WT2_OFF = 64 * K * OCG        # 139968
NW16 = 2 * 64 * K * OCG       # weight blob: 279936 f16 elements
BO1_OFF = 0
BO2_OFF = OCG
WD1_OFF = 2 * OCG
WD2_OFF = WD1_OFF + CPG * K * 64
GB_OFF = WD2_OFF + CPG * K * 64
N32 = GB_OFF + CPG * 4


def mkap(tile, off, dims):
    ap = tile[:]
    return bass.AP(tensor=ap.tensor, offset=ap.offset + off,
                   ap=[list(ap.ap[0])] + [list(d) for d in dims])


def dmkap(t_ap, off, dims):
    return bass.AP(tensor=t_ap.tensor, offset=t_ap.offset + off,
                   ap=[list(d) for d in dims])


ZBLK = 8192


def zero_dram(nc, zero_sb, dram_ap, rows, total):
    # stride-0 repeat DMAs corrupt data on this DMA engine; use one DMA per
    # block from a real zero tile instead.
    nblk = total // ZBLK
    rem = total - nblk * ZBLK
    for i in range(nblk):
        nc.sync.dma_start(out=dmkap(dram_ap, i * ZBLK, [[total, rows], [1, ZBLK]]),
                          in_=dmkap(zero_sb[:], 0, [[ZBLK, rows], [1, ZBLK]]))
    if rem:
        nc.sync.dma_start(out=dmkap(dram_ap, nblk * ZBLK, [[total, rows], [1, rem]]),
                          in_=dmkap(zero_sb[:], 0, [[ZBLK, rows], [1, rem]]))


def conv_phase(nc, tc, sfx, xpad_dram, wt_sb, bias_sb, off_dram):
    """27-tap conv: xpad_dram [64, B*10*PLANE] -> off_dram [81, B*D*3136]."""
    GUARD = 64
    CHUNKS = []
    for r0 in range(0, 58, 8):
        nr = min(8, 58 - r0)
        v0 = max(1, r0)
        v1 = min(57, r0 + nr)
        CHUNKS.append((r0 * 58, nr * 58, v0 - r0, v1 - v0))
    with tc.tile_pool(name=f"convp{sfx}", bufs=2) as pool, \
         tc.tile_pool(name=f"convps{sfx}", bufs=4, space="PSUM") as pspool:
        for b in range(B):
            for j in range(4):
                xpc = pool.tile([64, 2 * GUARD + 4 * PLANE], F32, tag="xpc")
                nc.vector.memset(xpc[:, :GUARD], 0.0)
                nc.vector.memset(xpc[:, GUARD + 4 * PLANE:], 0.0)
                nc.sync.dma_start(
                    out=xpc[:, GUARD:GUARD + 4 * PLANE],
                    in_=dmkap(xpad_dram[:], (b * 10 + 2 * j) * PLANE,
                              [[B * 10 * PLANE, 64], [1, 4 * PLANE]]))
                for ds in range(2):
                    d = 2 * j + ds
                    for (n0, nsz, vr, nv) in CHUNKS:
                        ps = pspool.tile([OCG, 512], F32, tag="cps")
                        for k in range(K):
                            kd, kh, kw = k // 9, (k // 3) % 3, k % 3
                            roff = GUARD + (ds + kd) * PLANE + (kh - 1) * 58 + (kw - 1) + n0
                            nc.tensor.matmul(ps[:, :nsz], wt_sb[:, k, :],
                                             mkap(xpc, roff, [[1, nsz]]),
                                             start=(k == 0), stop=(k == K - 1))
                        ot = pool.tile([OCG, 512], F32, tag="convot")
                        nc.vector.tensor_tensor(
                            out=ot[:, :nsz], in0=ps[:, :nsz],
                            in1=mkap(bias_sb, 0, [[0, nsz]]), op=ALU.add)
                        if nv <= 0:
                            continue
                        real_r0 = n0 // 58 + vr - 1
                        nc.sync.dma_start(
                            out=dmkap(off_dram[:], (b * D + d) * 3136 + real_r0 * 56,
                                      [[B * D * 3136, OCG], [1, nv * 56]]),
                            in_=mkap(ot, vr * 58 + 1, [[58, nv], [1, 56]]))


def dense_phase(nc, tc, sfx, xw_dram, off_dram, wd_sb, partial_dram, colsd_dram):
    """Dense 5^3 deform + einsum -> partial_dram [64, BV] (band-perm)."""
    with tc.tile_pool(name=f"densep{sfx}", bufs=1) as pool, \
         tc.tile_pool(name=f"densew{sfx}", bufs=1) as wpool, \
         tc.tile_pool(name=f"denseps{sfx}", bufs=2, space="PSUM") as pspool:
        for b in range(B):
            xw = pool.tile([P, XSZ], F32, tag="xw")
            for dd in range(XD):
                nc.sync.dma_start(
                    out=mkap(xw, dd * XH * XWW, [[1, 620]]),
                    in_=dmkap(xw_dram[:], b * XVOL + dd * 62 * 62,
                              [[BH * XWW, NB], [B * XVOL, CPG], [1, XH * XWW]]))
            for k in range(K):
                kd, kh, kw = k // 9 - 1, (k // 3) % 3 - 1, k % 3 - 1
                offt = pool.tile([P, 3, CH], F32, tag="offt")
                for ax in range(3):
                    for dd in range(D):
                        nc.sync.dma_start(
                            out=mkap(offt, ax * CH + dd * BH * W, [[1, BH * W]]),
                            in_=dmkap(off_dram[:],
                                      (3 * k + ax) * B * D * 3136 + (b * D + dd) * 3136,
                                      [[BH * W, NB], [0, CPG], [1, BH * W]]))
                nc.vector.tensor_scalar(out=offt[:], in0=offt[:], scalar1=CLAMP,
                                        scalar2=-CLAMP, op0=ALU.min, op1=ALU.max)
                hw = pool.tile([P, SS, CH], F32, tag="hw")
                for a in range(SS):
                    nc.scalar.activation(hw[:, a, :], offt[:, 2, :], ACTF.Abs,
                                         bias=float(-(a - 2)), scale=1.0)
                    nc.scalar.activation(hw[:, a, :], hw[:, a, :], ACTF.Relu,
                                         bias=1.0, scale=-1.0)
                cols = wpool.tile([P, CH], F32, tag="cols")
                pt = wpool.tile([P, CH], F32, tag="pt")
                at = wpool.tile([P, CH], F32, tag="at")
                tt = wpool.tile([P, CH], F32, tag="tt")
                hdsl = pool.tile([P, CH], F32, tag="hdsl")
                hhsl = pool.tile([P, CH], F32, tag="hhsl")
                first = True
                for sd in range(SS):
                    nc.scalar.activation(hdsl[:], offt[:, 0, :], ACTF.Abs,
                                         bias=float(-(sd - 2)), scale=1.0)
                    nc.scalar.activation(hdsl[:], hdsl[:], ACTF.Relu,
                                         bias=1.0, scale=-1.0)
                    for sh in range(SS):
                        nc.scalar.activation(hhsl[:], offt[:, 1, :], ACTF.Abs,
                                             bias=float(-(sh - 2)), scale=1.0)
                        nc.scalar.activation(hhsl[:], hhsl[:], ACTF.Relu,
                                             bias=1.0, scale=-1.0)
                        nc.vector.tensor_tensor(out=pt[:], in0=hdsl[:],
                                                in1=hhsl[:], op=ALU.mult)
                        for sw in range(SS):
                            xoff = ((1 + kd + sd) * XH * XWW + (1 + kh + sh) * XWW
                                    + (1 + kw + sw))
                            xap = mkap(xw, xoff, [[XH * XWW, D], [XWW, BH], [1, W]])
                            dst = at if sw == 0 else tt
                            nc.vector.tensor_tensor(out=dst[:], in0=xap,
                                                    in1=hw[:, sw, :], op=ALU.mult)
                            if sw > 0:
                                nc.vector.tensor_tensor(out=at[:], in0=at[:],
                                                        in1=tt[:], op=ALU.add)
                        if first:
                            nc.vector.tensor_tensor(out=cols[:], in0=pt[:], in1=at[:],
                                                    op=ALU.mult)
                            first = False
                        else:
                            nc.gpsimd.tensor_tensor(out=tt[:], in0=pt[:], in1=at[:],
                                                    op=ALU.mult)
                            nc.gpsimd.tensor_tensor(out=cols[:], in0=cols[:], in1=tt[:],
                                                    op=ALU.add)
                nc.sync.dma_start(
                    out=dmkap(colsd_dram[:], (b * K + k) * CH,
                              [[B * K * CH, P], [1, CH]]),
                    in_=cols[:])
            tc.strict_bb_all_engine_barrier()
            for hb in range(NB):
                ps2 = pspool.tile([64, 2048], F32, tag="eps")
                for k in range(K):
                    cr = wpool.tile([CPG, CH], F32, tag="colsr")
                    nc.sync.dma_start(
                        out=cr[:],
                        in_=dmkap(colsd_dram[:], hb * CPG * B * K * CH + (b * K + k) * CH,
                                  [[B * K * CH, CPG], [1, CH]]))
                    for i in range(4):
                        nc.tensor.matmul(ps2[:, i * 512:i * 512 + 448], wd_sb[:, k, :],
                                         cr[:, i * 448:(i + 1) * 448],
                                         start=(k == 0), stop=(k == K - 1))
                pot = wpool.tile([64, CH], F32, tag="pot")
                nc.vector.tensor_copy(out=pot[:], in_=mkap(ps2, 0, [[512, 4], [1, 448]]))
                nc.sync.dma_start(
                    out=dmkap(partial_dram[:], b * V + hb * CH, [[BV, 64], [1, CH]]),
                    in_=pot[:])


def ensure_consts(nc):
    for v in (2.0, -2.0, -1.0, 1e-5):
        key = (F32, v)
        if key not in nc.const_aps.aps:
            t = nc.alloc_sbuf_tensor(f"const-f32-{v}", [128, 1], F32)
            nc.gpsimd.memset(t.ap(), v)
            nc.const_aps.aps[key] = t.ap()


def bn_stats8(nc, tc, pool, sfx, src_dram, gamma_sb, beta_sb):
    """BN scale/shift for [CPG, BV] shard (band layout is irrelevant)."""
    sum_t = pool.tile([CPG, 1], F32, tag=f"bnsum{sfx}")
    sq_t = pool.tile([CPG, 1], F32, tag=f"bnsq{sfx}")
    t1 = pool.tile([CPG, 1], F32, tag=f"bnt1{sfx}")
    t2 = pool.tile([CPG, 1], F32, tag=f"bnt2{sfx}")
    with tc.tile_pool(name=f"bnstat{sfx}", bufs=1) as big:
        for i in range(NCHK):
            ht = big.tile([CPG, CSZ], F32, tag=f"bnh{sfx}")
            sqv = big.tile([CPG, CSZ], F32, tag=f"bnsqv{sfx}")
            nc.sync.dma_start(out=ht[:], in_=dmkap(src_dram, i * CSZ, [[BV, CPG], [1, CSZ]]))
            nc.vector.tensor_reduce(out=t1[:], in_=ht[:], axis=AX.X, op=ALU.add)
            nc.vector.tensor_tensor(out=sqv[:], in0=ht[:], in1=ht[:], op=ALU.mult)
            nc.vector.tensor_reduce(out=t2[:], in_=sqv[:], axis=AX.X, op=ALU.add)
            if i == 0:
                nc.vector.tensor_copy(out=sum_t[:], in_=t1[:])
                nc.vector.tensor_copy(out=sq_t[:], in_=t2[:])
            else:
                nc.vector.tensor_tensor(out=sum_t[:], in0=sum_t[:], in1=t1[:], op=ALU.add)
                nc.vector.tensor_tensor(out=sq_t[:], in0=sq_t[:], in1=t2[:], op=ALU.add)
    N = float(BV)
    scale = pool.tile([CPG, 1], F32, tag=f"bnscale{sfx}")
    shift = pool.tile([CPG, 1], F32, tag=f"bnshift{sfx}")
    mean = t1
    nc.vector.tensor_scalar(out=mean[:], in0=sum_t[:], scalar1=1.0 / N, scalar2=0.0,
                            op0=ALU.mult, op1=ALU.add)
    var = t2
    nc.vector.tensor_scalar(out=var[:], in0=sq_t[:], scalar1=1.0 / N, scalar2=0.0,
                            op0=ALU.mult, op1=ALU.add)
    msq = pool.tile([CPG, 1], F32, tag=f"bnmsq{sfx}")
    nc.vector.tensor_tensor(out=msq[:], in0=mean[:], in1=mean[:], op=ALU.mult)
    nc.vector.tensor_tensor(out=var[:], in0=var[:], in1=msq[:], op=ALU.subtract)
    rstd = pool.tile([CPG, 1], F32, tag=f"bnrstd{sfx}")
    nc.scalar.activation(out=rstd[:], in_=var[:], func=ACTF.Sqrt, bias=1e-5, scale=1.0)
    nc.vector.reciprocal(out=rstd[:], in_=rstd[:])
    nc.vector.tensor_tensor(out=scale[:], in0=gamma_sb[:], in1=rstd[:], op=ALU.mult)
    nc.vector.tensor_tensor(out=shift[:], in0=mean[:], in1=scale[:], op=ALU.mult)
    nc.vector.tensor_tensor(out=shift[:], in0=beta_sb[:], in1=shift[:], op=ALU.subtract)
    return scale, shift


NCHK = 4
CSZ = BV // NCHK


def deband_store(nc, src_tile, dst_dram, i):
    """Store band-layout chunk i of [CPG, CSZ] to canonical [CPG, BV] DRAM."""
    b, half = i // 2, i % 2
    for hbr in range(7):
        hb = half * 7 + hbr
        nc.sync.dma_start(
            out=dmkap(dst_dram, b * V + hb * BH * W, [[BV, CPG], [3136, D], [1, BH * W]]),
            in_=mkap(src_tile, hbr * CH, [[BH * W, D], [1, BH * W]]))


def band_load(nc, dst_tile, src_dram, i):
    """Load canonical [CPG, BV] DRAM into band-layout chunk i [CPG, CSZ]."""
    b, half = i // 2, i % 2
    for hbr in range(7):
        hb = half * 7 + hbr
        nc.sync.dma_start(
            out=mkap(dst_tile, hbr * CH, [[BH * W, D], [1, BH * W]]),
            in_=dmkap(src_dram, b * V + hb * BH * W, [[BV, CPG], [3136, D], [1, BH * W]]))


# ------------------------------------------------------------ program nc --
def build_fused(debug=False):
    nc = bass.Bass("TRN2", target_bir_lowering=False, num_devices=NCORES)
    ensure_consts(nc)
    px16_in = nc.declare_dram_parameter("px16", [1, NX16], F16, isOutput=False)
    pw16_in = nc.declare_dram_parameter("pw16", [1, NW16], F16, isOutput=False)
    pk32_in = nc.declare_dram_parameter("pk32", [1, N32], F32, isOutput=False)
    out_d = nc.declare_dram_parameter("out", [64, BV], F16, isOutput=True)
    og16_d = nc.dram_tensor("og16", [CPG, BV], F16)
    ofull16_d = nc.dram_tensor("ofull16", [64, BV], F16, addr_space="Shared")

    xs32_d = nc.dram_tensor("xs32", [CPG, BV], F32)
    xb16_d = nc.dram_tensor("xb16", [CPG, BV], F16)
    xfull16_d = nc.dram_tensor("xfull16", [64, BV], F16, addr_space="Shared")
    xfull_d = nc.dram_tensor("xfull", [64, BV], F32)
    xpad_d = nc.dram_tensor("xpad", [64, B * 10 * PLANE], F32)
    xw_d = nc.dram_tensor("xw", [CPG, B * XVOL], F32)
    xw2_d = nc.dram_tensor("xw2", [CPG, B * XVOL], F32)
    hpad_d = nc.dram_tensor("hpad", [64, B * 10 * PLANE], F32)
    off_d = nc.dram_tensor("offs", [OCG, B * D * 3136], F32)
    colsd_d = nc.dram_tensor("colsd", [P, B * K * CH], F32)
    part_d = nc.dram_tensor("part", [64, BV], F32)
    h1s_d = nc.dram_tensor("h1s", [CPG, BV], F32)
    h2s_d = nc.dram_tensor("h2s", [CPG, BV], F32)
    hg1_d = nc.dram_tensor("hg1", [CPG, BV], F32)
    hfull_d = nc.dram_tensor("hfull", [64, BV], F32, addr_space="Shared")

    dbg = {}
    if debug:
        for nm, shp in (("dxfull", [64, BV]), ("doff1", [OCG, BV]),
                        ("dpart1", [64, BV]), ("dh1s", [CPG, BV]),
                        ("dhg1", [CPG, BV]), ("dhfull", [64, BV]),
                        ("doff2", [OCG, BV]), ("dpart2", [64, BV]),
                        ("dh2s", [CPG, BV]), ("dxpad", [64, B * 10 * PLANE]),
                        ("dxw", [CPG, B * XVOL])):
            dbg[nm] = nc.declare_dram_parameter(nm, shp, F32, isOutput=True)

    def dump(nm, src, rows, total):
        if not debug:
            return
        nc.sync.dma_start(out=dmkap(dbg[nm][:], 0, [[total, rows], [1, total]]),
                          in_=dmkap(src[:], 0, [[total, rows], [1, total]]))

    with TileContext(nc) as tc:
        with tc.tile_pool(name="single", bufs=1) as sp:
            wt1_sb = sp.tile([64, K, OCG], F32, tag="wt1")
            wt2_sb = sp.tile([64, K, OCG], F32, tag="wt2")
            for woff, wsb, wtag in ((WT1_OFF, wt1_sb, "w16a"), (WT2_OFF, wt2_sb, "w16b")):
                w16 = sp.tile([64, K * OCG], F16, tag=wtag)
                nc.sync.dma_start(
                    out=w16[:],
                    in_=dmkap(pw16_in[:], woff, [[K * OCG, 64], [1, K * OCG]]))
                nc.vector.tensor_copy(out=wsb[:], in_=w16[:])
            bo1_sb = sp.tile([OCG, 1], F32, tag="bo1")
            nc.sync.dma_start(out=bo1_sb[:],
                              in_=dmkap(pk32_in[:], BO1_OFF, [[1, OCG], [1, 1]]))
            bo2_sb = sp.tile([OCG, 1], F32, tag="bo2")
            nc.sync.dma_start(out=bo2_sb[:],
                              in_=dmkap(pk32_in[:], BO2_OFF, [[1, OCG], [1, 1]]))
            wd1_sb = sp.tile([CPG, K, 64], F32, tag="wd1")
            nc.sync.dma_start(out=wd1_sb[:],
                              in_=dmkap(pk32_in[:], WD1_OFF, [[K * 64, CPG], [1, K * 64]]))
            wd2_sb = sp.tile([CPG, K, 64], F32, tag="wd2")
            nc.sync.dma_start(out=wd2_sb[:],
                              in_=dmkap(pk32_in[:], WD2_OFF, [[K * 64, CPG], [1, K * 64]]))
            gb_sb = sp.tile([CPG, 4], F32, tag="gb")
            nc.sync.dma_start(out=gb_sb[:],
                              in_=dmkap(pk32_in[:], GB_OFF, [[4, CPG], [1, 4]]))
            # zero all padded scratch volumes up front
            with tc.tile_pool(name="zpool", bufs=1) as zp:
                zero_sb = zp.tile([64, ZBLK], F32, tag="zsb")
                nc.vector.memset(zero_sb[:], 0.0)
                zero_dram(nc, zero_sb, xpad_d[:], 64, B * 10 * PLANE)
                zero_dram(nc, zero_sb, hpad_d[:], 64, B * 10 * PLANE)
                zero_dram(nc, zero_sb, xw_d[:], CPG, B * XVOL)
                zero_dram(nc, zero_sb, xw2_d[:], CPG, B * XVOL)

            # AllGather x slices (f16) -> xfull16, then cast passes
            nc.gpsimd.dma_start(
                out=xb16_d[:],
                in_=dmkap(px16_in[:], XS_OFF, [[BV, CPG], [1, BV]]))
            nc.gpsimd.collective_compute(
                "AllGather", ALU.bypass, replica_groups=RG,
                ins=[xb16_d[:].opt()], outs=[xfull16_d[:].opt()])
            # xs cast to f32 (local slice; overlaps with the collective)
            with tc.tile_pool(name="xcast", bufs=2) as cp:
                for i in range(NCHK):
                    a16 = cp.tile([CPG, CSZ], F16, tag="a16")
                    nc.sync.dma_start(
                        out=a16[:],
                        in_=dmkap(px16_in[:], XS_OFF + i * CSZ, [[BV, CPG], [1, CSZ]]))
                    a32 = cp.tile([CPG, CSZ], F32, tag="a32")
                    nc.vector.tensor_copy(out=a32[:], in_=a16[:])
                    nc.sync.dma_start(
                        out=dmkap(xs32_d[:], i * CSZ, [[BV, CPG], [1, CSZ]]),
                        in_=a32[:])
            tc.strict_bb_all_engine_barrier()
            # xfull cast to f32
            with tc.tile_pool(name="xfcast", bufs=2) as cp:
                for i in range(NCHK):
                    a16 = cp.tile([64, CSZ], F16, tag="b16")
                    nc.sync.dma_start(
                        out=a16[:],
                        in_=dmkap(xfull16_d[:], i * CSZ, [[BV, 64], [1, CSZ]]))
                    a32 = cp.tile([64, CSZ], F32, tag="b32")
                    nc.vector.tensor_copy(out=a32[:], in_=a16[:])
                    nc.sync.dma_start(
                        out=dmkap(xfull_d[:], i * CSZ, [[BV, 64], [1, CSZ]]),
                        in_=a32[:])
            tc.strict_bb_all_engine_barrier()
            dump("dxfull", xfull_d, 64, BV)

            # interior fills: xpad <- xfull, xw <- xs32
            for b in range(B):
                for d in range(D):
                    nc.sync.dma_start(
                        out=dmkap(xpad_d[:], (b * 10 + d + 1) * PLANE + 59,
                                  [[B * 10 * PLANE, 64], [58, 56], [1, 56]]),
                        in_=dmkap(xfull_d[:], b * V + d * 3136,
                                  [[BV, 64], [56, 56], [1, 56]]))
                    nc.sync.dma_start(
                        out=dmkap(xw_d[:], b * XVOL + (d + 3) * 3844 + 3 * 62 + 3,
                                  [[B * XVOL, CPG], [62, 56], [1, 56]]),
                        in_=dmkap(xs32_d[:], b * V + d * 3136,
                                  [[BV, CPG], [56, 56], [1, 56]]))
            tc.strict_bb_all_engine_barrier()

            dump("dxpad", xpad_d, 64, B * 10 * PLANE)
            dump("dxw", xw_d, CPG, B * XVOL)
            tc.strict_bb_all_engine_barrier()
            # ---- layer 1
            conv_phase(nc, tc, "1", xpad_d, wt1_sb, bo1_sb, off_d)
            tc.strict_bb_all_engine_barrier()
            dump("doff1", off_d, OCG, BV)
            dense_phase(nc, tc, "1", xw_d, off_d, wd1_sb, part_d, colsd_d)
            tc.strict_bb_all_engine_barrier()
            dump("dpart1", part_d, 64, BV)
            nc.gpsimd.collective_compute(
                "ReduceScatter", ALU.add, replica_groups=RG,
                ins=[part_d[:].opt()], outs=[h1s_d[:].opt()])
            tc.strict_bb_all_engine_barrier()
            dump("dh1s", h1s_d, CPG, BV)

            # ---- BN1 + relu -> hg1 (canonical), then AllGather -> hfull
            with tc.tile_pool(name="bn1p", bufs=1) as pool:
                scale, shift = bn_stats8(nc, tc, pool, "a", h1s_d[:], gb_sb[:, 0:1],
                                         gb_sb[:, 1:2])
                with tc.tile_pool(name="bn1ap", bufs=2) as apool:
                    for i in range(NCHK):
                        ht = apool.tile([CPG, CSZ], F32, tag="bnh1")
                        nc.sync.dma_start(
                            out=ht[:], in_=dmkap(h1s_d[:], i * CSZ, [[BV, CPG], [1, CSZ]]))
                        nc.scalar.activation(out=ht[:], in_=ht[:], func=ACTF.Relu,
                                             bias=shift[:], scale=scale[:])
                        deband_store(nc, ht, hg1_d[:], i)
            tc.strict_bb_all_engine_barrier()
            dump("dhg1", hg1_d, CPG, BV)
            nc.gpsimd.collective_compute(
                "AllGather", ALU.bypass, replica_groups=RG,
                ins=[hg1_d[:].opt()], outs=[hfull_d[:].opt()])
            tc.strict_bb_all_engine_barrier()
            dump("dhfull", hfull_d, 64, BV)

            # interior fills: hpad <- hfull, xw2 <- hg1
            for b in range(B):
                for d in range(D):
                    nc.sync.dma_start(
                        out=dmkap(hpad_d[:], (b * 10 + d + 1) * PLANE + 59,
                                  [[B * 10 * PLANE, 64], [58, 56], [1, 56]]),
                        in_=dmkap(hfull_d[:], b * V + d * 3136,
                                  [[BV, 64], [56, 56], [1, 56]]))
                    nc.sync.dma_start(
                        out=dmkap(xw2_d[:], b * XVOL + (d + 3) * 3844 + 3 * 62 + 3,
                                  [[B * XVOL, CPG], [62, 56], [1, 56]]),
                        in_=dmkap(hg1_d[:], b * V + d * 3136,
                                  [[BV, CPG], [56, 56], [1, 56]]))
            tc.strict_bb_all_engine_barrier()

            # ---- layer 2
            conv_phase(nc, tc, "2", hpad_d, wt2_sb, bo2_sb, off_d)
            tc.strict_bb_all_engine_barrier()
            dump("doff2", off_d, OCG, BV)
            dense_phase(nc, tc, "2", xw2_d, off_d, wd2_sb, part_d, colsd_d)
            tc.strict_bb_all_engine_barrier()
            dump("dpart2", part_d, 64, BV)
            nc.gpsimd.collective_compute(
                "ReduceScatter", ALU.add, replica_groups=RG,
                ins=[part_d[:].opt()], outs=[h2s_d[:].opt()])
            tc.strict_bb_all_engine_barrier()
            dump("dh2s", h2s_d, CPG, BV)

            # ---- BN2 + residual + relu -> out (canonical)
            with tc.tile_pool(name="bn2p", bufs=1) as pool:
                scale, shift = bn_stats8(nc, tc, pool, "b", h2s_d[:], gb_sb[:, 2:3],
                                         gb_sb[:, 3:4])
                with tc.tile_pool(name="bn2ap", bufs=1) as apool:
                    for i in range(NCHK):
                        ht = apool.tile([CPG, CSZ], F32, tag="bnh2")
                        rt = apool.tile([CPG, CSZ], F32, tag="bnr2")
                        nc.sync.dma_start(
                            out=ht[:], in_=dmkap(h2s_d[:], i * CSZ, [[BV, CPG], [1, CSZ]]))
                        band_load(nc, rt, xs32_d[:], i)
                        nc.vector.tensor_tensor(out=ht[:], in0=ht[:],
                                                in1=mkap(scale, 0, [[0, CSZ]]), op=ALU.mult)
                        nc.vector.tensor_tensor(out=ht[:], in0=ht[:],
                                                in1=mkap(shift, 0, [[0, CSZ]]), op=ALU.add)
                        nc.vector.tensor_tensor(out=ht[:], in0=ht[:], in1=rt[:], op=ALU.add)
                        ht16 = apool.tile([CPG, CSZ], F16, tag="bnh2c")
                        nc.vector.tensor_scalar(out=ht16[:], in0=ht[:], scalar1=0.0,
                                                scalar2=0.0, op0=ALU.max, op1=ALU.add)
                        deband_store(nc, ht16, og16_d[:], i)
            # replicate the output on every core so the host fetches 1 shard
            tc.strict_bb_all_engine_barrier()
            nc.gpsimd.collective_compute(
                "AllGather", ALU.bypass, replica_groups=RG,
                ins=[og16_d[:].opt()], outs=[ofull16_d[:].opt()])
            tc.strict_bb_all_engine_barrier()
            nc.sync.dma_start(out=out_d[:], in_=ofull16_d[:])
    return nc


# ---------------------------------------------------------- cached runner --
class Runner:
    """jit(shard_map(bass_exec)) built once; later calls only move data."""

    def __init__(self, nc, n_cores=NCORES):
        _b2j.install_neuronx_cc_hook()
        self.n_cores = n_cores
        partition_name = nc.partition_id_tensor.name if nc.partition_id_tensor else None
        in_names, out_names, out_avals, zero_shapes = [], [], [], []
        for alloc in nc.m.functions[0].allocations:
            if not isinstance(alloc, mybir.MemoryLocationSet):
                continue
            name = alloc.memorylocations[0].name
            if alloc.kind == "ExternalInput":
                if name != partition_name:
                    in_names.append(name)
            elif alloc.kind == "ExternalOutput":
                shape = tuple(alloc.tensor_shape)
                dtype = mybir.dt.np(alloc.dtype)
                out_names.append(name)
                out_avals.append(jax.core.ShapedArray(shape, dtype))
                zero_shapes.append((shape, dtype))
        self.n_params = len(in_names)
        self.in_names = list(in_names)
        self.out_names = out_names
        self.out_avals = out_avals
        all_in_names = list(in_names)
        if partition_name is not None:
            all_in_names.append(partition_name)

        def _body(*args):
            operands = list(args)
            if partition_name is not None:
                operands.append(_b2j.partition_id_tensor())
            outs = _b2j._bass_exec_p.bind(
                *operands,
                out_avals=tuple(out_avals),
                in_names=tuple(all_in_names),
                out_names=tuple(out_names),
                lowering_input_output_aliases=(),
                sim_require_finite=True,
                sim_require_nnan=True,
                nc=nc,
            )
            return tuple(outs)

        devices = jax.devices()[:n_cores]
        assert len(devices) == n_cores
        self.mesh = Mesh(np.asarray(devices), ("core",))
        self.fn = jax.jit(
            shard_map(_body, mesh=self.mesh,
                      in_specs=(PartitionSpec("core"),) * self.n_params,
                      out_specs=(PartitionSpec("core"),) * len(out_names),
                      check_rep=False),
            keep_unused=True,
        )

    def __call__(self, in_maps):
        concat_in = [
            np.concatenate([np.asarray(m[name]) for m in in_maps], axis=0)
            for name in self.in_names
        ]
        out_arrs = self.fn(*concat_in)
        outs = [np.asarray(a) for a in out_arrs]
        return [
            {
                name: outs[i].reshape(self.n_cores, *self.out_avals[i].shape)[c]
                for i, name in enumerate(self.out_names)
            }
            for c in range(self.n_cores)
        ]


_RUNNER = None


def _get_runner():
    global _RUNNER
    if _RUNNER is None:
        _RUNNER = Runner(build_fused())
    return _RUNNER


# ----------------------------------------------------------------- kernel --
def make_inmaps(inputs):
    x = np.ascontiguousarray(inputs["x"], dtype=np.float32)
    xt = np.ascontiguousarray(x.transpose(1, 0, 2, 3, 4)).reshape(64, BV)

    def wslices(w_off, b_off, w_dc):
        wts, bs, wds = [], [], []
        w_off = np.asarray(w_off, np.float32).reshape(G * OCG, 64, K)
        w_dc = np.asarray(w_dc, np.float32).reshape(64, G, CPG, K)
        b_off = np.asarray(b_off, np.float32)
        for g in range(G):
            wts.append(np.ascontiguousarray(
                w_off[g * OCG:(g + 1) * OCG].transpose(1, 2, 0)).reshape(64, -1))
            bs.append(np.ascontiguousarray(b_off[g * OCG:(g + 1) * OCG]).reshape(OCG, 1))
            wds.append(np.ascontiguousarray(
                w_dc[:, g].transpose(1, 2, 0)).reshape(CPG, -1))
        return wts, bs, wds

    wt1, bo1, wd1 = wslices(inputs["w_off1"], inputs["b_off1"], inputs["w_dc1"])
    wt2, bo2, wd2 = wslices(inputs["w_off2"], inputs["b_off2"], inputs["w_dc2"])
    g1 = np.asarray(inputs["gamma1"], np.float32)
    b1 = np.asarray(inputs["beta1"], np.float32)
    g2 = np.asarray(inputs["gamma2"], np.float32)
    b2 = np.asarray(inputs["beta2"], np.float32)

    px16 = np.empty((G, NX16), np.float16)
    pw16 = np.empty((G, NW16), np.float16)
    pk32 = np.empty((G, N32), np.float32)
    for g in range(G):
        sl = slice(g * CPG, (g + 1) * CPG)
        gb = np.stack([g1[sl], b1[sl], g2[sl], b2[sl]], axis=1).astype(np.float32)
        px16[g, XS_OFF:XS_OFF + CPG * BV] = xt[sl].reshape(-1)
        pw16[g, WT1_OFF:WT1_OFF + 64 * K * OCG] = wt1[g].reshape(-1)
        pw16[g, WT2_OFF:WT2_OFF + 64 * K * OCG] = wt2[g].reshape(-1)
        pk32[g, BO1_OFF:BO1_OFF + OCG] = bo1[g].reshape(-1)
        pk32[g, BO2_OFF:BO2_OFF + OCG] = bo2[g].reshape(-1)
        pk32[g, WD1_OFF:WD1_OFF + CPG * K * 64] = wd1[g].reshape(-1)
        pk32[g, WD2_OFF:WD2_OFF + CPG * K * 64] = wd2[g].reshape(-1)
        pk32[g, GB_OFF:GB_OFF + CPG * 4] = gb.reshape(-1)
    return px16, pw16, pk32


_WCACHE = {"pw16": None, "pk32": None, "dev": None}


def kernel(**inputs):
    runner = _get_runner()
    px16, pw16, pk32 = make_inmaps(inputs)
    # weights live on device across calls; re-ship only when they change
    if (_WCACHE["dev"] is None
            or not np.array_equal(_WCACHE["pw16"], pw16)
            or not np.array_equal(_WCACHE["pk32"], pk32)):
        shard = NamedSharding(runner.mesh, PartitionSpec("core"))
        dev = (jax.device_put(pw16, shard), jax.device_put(pk32, shard))
        jax.block_until_ready(dev)
        _WCACHE.update(pw16=pw16, pk32=pk32, dev=dev)
    dev_pw16, dev_pk32 = _WCACHE["dev"]
    args = {"px16": px16, "pw16": dev_pw16, "pk32": dev_pk32}
    out_arrs = runner.fn(*[args[n] for n in runner.in_names])
    oi = runner.out_names.index("out")
    # output is AllGather-replicated across cores; fetch a single shard
    out = np.asarray(out_arrs[oi].addressable_shards[0].data)  # [64, BV] f16
    return np.ascontiguousarray(
        out.reshape(64, B, D, H, W).transpose(1, 0, 2, 3, 4)).astype(np.float32)


# revision 19
# speedup vs baseline: 1.2587x; 1.0447x over previous
"""Trainium2 Bass kernel for nn_DeformBasicBlock1 (deformable conv block).

Fully fused single-invocation SPMD program over 8 cores, group-sharded:
core g owns channel-group g (8 x-channels / 81 offset channels / 8 output
channels).  Cross-core exchange happens on device:
  AllGather(x slices) -> conv1 -> deform1 -> ReduceScatter(partials)
  -> per-channel BN1+relu -> AllGather -> conv2 -> deform2
  -> ReduceScatter -> BN2 + residual + relu -> per-core output slice.
The jitted executable is built once and cached in module globals; repeat
kernel() calls only move input/output slices (~35 MB) over the wire.
"""
import json
import numpy as np

import jax
import jax.numpy as jnp
from jax.sharding import Mesh, PartitionSpec, NamedSharding
from jax.experimental.shard_map import shard_map

import concourse.bass as bass
import concourse.mybir as mybir
from concourse.tile import TileContext
import concourse.bass_utils as bass_utils
import concourse.tile_utils as tile_utils

# ---------------------------------------------------------------- tilefix --
_orig_compile_bir_kernel = bass_utils.compile_bir_kernel


def _split_waits_json(bir_json: bytes) -> bytes:
    j = json.loads(bir_json)
    ctr = 0
    changed = False
    for f in j["functions"]:
        for b in f["blocks"]:
            insts = b["instructions"]
            if not any(
                len((i.get("sync_info") or {}).get("on_wait") or []) > 1
                for i in insts
            ):
                continue
            changed = True
            out = []
            for inst in insts:
                si = inst.get("sync_info")
                if si:
                    ow = si.get("on_wait") or []
                    if len(ow) > 1:
                        for w in ow[:-1]:
                            ctr += 1
                            nop = {
                                "engine": inst["engine"],
                                "ins": [],
                                "outs": [],
                                "name": f"WSPLIT-{ctr}",
                                "opcode": "NoOp",
                                "sync_info": {"on_update": [], "on_wait": [w]},
                            }
                            if "debug" in inst:
                                nop["debug"] = inst["debug"]
                            out.append(nop)
                        si["on_wait"] = [ow[-1]]
                out.append(inst)
            b["instructions"] = out
    return json.dumps(j).encode() if changed else bir_json


def _patched_compile_bir_kernel(bir_json, tmpdir, neff_name="file.neff"):
    if isinstance(bir_json, str):
        bir_json = bir_json.encode()
    return _orig_compile_bir_kernel(_split_waits_json(bir_json), tmpdir, neff_name)


bass_utils.compile_bir_kernel = _patched_compile_bir_kernel
import concourse.bass2jax as _b2j  # noqa: E402

_b2j.compile_bir_kernel = _patched_compile_bir_kernel
try:
    tile_utils.max_sbuf_usage = 204 * 1024
except Exception:
    pass

# ------------------------------------------------------------- constants --
B, D, H, W = 2, 8, 56, 56
CPG, G, K = 8, 8, 27
OCG = 81
V = D * H * W
BV = B * V
PLANE = 3364  # 58*58
NB, BH = 14, 4
P = NB * CPG  # 112
CH = D * BH * W  # 1792
XD, XH, XWW = 14, 10, 62
XSZ = XD * XH * XWW
XVOL = XD * 62 * 62
SS = 5
CLAMP = 1.999
F32 = mybir.dt.float32
F16 = mybir.dt.float16
AX = mybir.AxisListType
ALU = mybir.AluOpType
ACTF = mybir.ActivationFunctionType
NCORES = 8
RG = [list(range(NCORES))]

# packed-input blob layout (per core)
XS_OFF = 0
NX16 = CPG * BV               # x blob: 401408 f16 elements
WT1_OFF = 0
WT2_OFF = 64 * K * OCG        # 139968
NW16 = 2 * 64 * K * OCG       # weight blob: 279936 f16 elements
BO1_OFF = 0
BO2_OFF = OCG
WD1_OFF = 2 * OCG
WD2_OFF = WD1_OFF + CPG * K * 64
GB_OFF = WD2_OFF + CPG * K * 64
N32 = GB_OFF + CPG * 4


def mkap(tile, off, dims):
    ap = tile[:]
    return bass.AP(tensor=ap.tensor, offset=ap.offset + off,
                   ap=[list(ap.ap[0])] + [list(d) for d in dims])


def dmkap(t_ap, off, dims):
    return bass.AP(tensor=t_ap.tensor, offset=t_ap.offset + off,
                   ap=[list(d) for d in dims])


ZBLK = 8192


def zero_dram(nc, zero_sb, dram_ap, rows, total):
    # stride-0 repeat DMAs corrupt data on this DMA engine; use one DMA per
    # block from a real zero tile instead.
    nblk = total // ZBLK
    rem = total - nblk * ZBLK
    for i in range(nblk):
        nc.sync.dma_start(out=dmkap(dram_ap, i * ZBLK, [[total, rows], [1, ZBLK]]),
                          in_=dmkap(zero_sb[:], 0, [[ZBLK, rows], [1, ZBLK]]))
    if rem:
        nc.sync.dma_start(out=dmkap(dram_ap, nblk * ZBLK, [[total, rows], [1, rem]]),
                          in_=dmkap(zero_sb[:], 0, [[ZBLK, rows], [1, rem]]))


def conv_phase(nc, tc, sfx, xpad_dram, wt_sb, bias_sb, off_dram):
    """27-tap conv: xpad_dram [64, B*10*PLANE] -> off_dram [81, B*D*3136]."""
    GUARD = 64
    CHUNKS = []
    for r0 in range(0, 58, 8):
        nr = min(8, 58 - r0)
        v0 = max(1, r0)
        v1 = min(57, r0 + nr)
        CHUNKS.append((r0 * 58, nr * 58, v0 - r0, v1 - v0))
    with tc.tile_pool(name=f"convp{sfx}", bufs=2) as pool, \
         tc.tile_pool(name=f"convps{sfx}", bufs=4, space="PSUM") as pspool:
        for b in range(B):
            for j in range(4):
                xpc = pool.tile([64, 2 * GUARD + 4 * PLANE], F32, tag="xpc")
                nc.vector.memset(xpc[:, :GUARD], 0.0)
                nc.vector.memset(xpc[:, GUARD + 4 * PLANE:], 0.0)
                nc.sync.dma_start(
                    out=xpc[:, GUARD:GUARD + 4 * PLANE],
                    in_=dmkap(xpad_dram[:], (b * 10 + 2 * j) * PLANE,
                              [[B * 10 * PLANE, 64], [1, 4 * PLANE]]))
                for ds in range(2):
                    d = 2 * j + ds
                    for (n0, nsz, vr, nv) in CHUNKS:
                        ps = pspool.tile([OCG, 512], F32, tag="cps")
                        for k in range(K):
                            kd, kh, kw = k // 9, (k // 3) % 3, k % 3
                            roff = GUARD + (ds + kd) * PLANE + (kh - 1) * 58 + (kw - 1) + n0
                            nc.tensor.matmul(ps[:, :nsz], wt_sb[:, k, :],
                                             mkap(xpc, roff, [[1, nsz]]),
                                             start=(k == 0), stop=(k == K - 1))
                        ot = pool.tile([OCG, 512], F32, tag="convot")
                        nc.vector.tensor_tensor(
                            out=ot[:, :nsz], in0=ps[:, :nsz],
                            in1=mkap(bias_sb, 0, [[0, nsz]]), op=ALU.add)
                        if nv <= 0:
                            continue
                        real_r0 = n0 // 58 + vr - 1
                        nc.sync.dma_start(
                            out=dmkap(off_dram[:], (b * D + d) * 3136 + real_r0 * 56,
                                      [[B * D * 3136, OCG], [1, nv * 56]]),
                            in_=mkap(ot, vr * 58 + 1, [[58, nv], [1, 56]]))


def dense_phase(nc, tc, sfx, xw_dram, off_dram, wd_sb, partial_dram, colsd_dram):
    """Dense 5^3 deform + einsum -> partial_dram [64, BV] (band-perm)."""
    with tc.tile_pool(name=f"densep{sfx}", bufs=1) as pool, \
         tc.tile_pool(name=f"densew{sfx}", bufs=1) as wpool, \
         tc.tile_pool(name=f"denseps{sfx}", bufs=2, space="PSUM") as pspool:
        for b in range(B):
            xw = pool.tile([P, XSZ], F32, tag="xw")
            for dd in range(XD):
                nc.sync.dma_start(
                    out=mkap(xw, dd * XH * XWW, [[1, 620]]),
                    in_=dmkap(xw_dram[:], b * XVOL + dd * 62 * 62,
                              [[BH * XWW, NB], [B * XVOL, CPG], [1, XH * XWW]]))
            for k in range(K):
                kd, kh, kw = k // 9 - 1, (k // 3) % 3 - 1, k % 3 - 1
                offt = pool.tile([P, 3, CH], F32, tag="offt")
                for ax in range(3):
                    for dd in range(D):
                        nc.sync.dma_start(
                            out=mkap(offt, ax * CH + dd * BH * W, [[1, BH * W]]),
                            in_=dmkap(off_dram[:],
                                      (3 * k + ax) * B * D * 3136 + (b * D + dd) * 3136,
                                      [[BH * W, NB], [0, CPG], [1, BH * W]]))
                nc.vector.tensor_scalar(out=offt[:], in0=offt[:], scalar1=CLAMP,
                                        scalar2=-CLAMP, op0=ALU.min, op1=ALU.max)
                hw = pool.tile([P, SS, CH], F32, tag="hw")
                for a in range(SS):
                    nc.scalar.activation(hw[:, a, :], offt[:, 2, :], ACTF.Abs,
                                         bias=float(-(a - 2)), scale=1.0)
                    nc.scalar.activation(hw[:, a, :], hw[:, a, :], ACTF.Relu,
                                         bias=1.0, scale=-1.0)
                cols = wpool.tile([P, CH], F32, tag="cols")
                pt = wpool.tile([P, CH], F32, tag="pt")
                at = wpool.tile([P, CH], F32, tag="at")
                tt = wpool.tile([P, CH], F32, tag="tt")
                hdsl = pool.tile([P, CH], F32, tag="hdsl")
                hhsl = pool.tile([P, CH], F32, tag="hhsl")
                first = True
                for sd in range(SS):
                    nc.scalar.activation(hdsl[:], offt[:, 0, :], ACTF.Abs,
                                         bias=float(-(sd - 2)), scale=1.0)
                    nc.scalar.activation(hdsl[:], hdsl[:], ACTF.Relu,
                                         bias=1.0, scale=-1.0)
                    for sh in range(SS):
                        nc.scalar.activation(hhsl[:], offt[:, 1, :], ACTF.Abs,
                                             bias=float(-(sh - 2)), scale=1.0)
                        nc.scalar.activation(hhsl[:], hhsl[:], ACTF.Relu,
                                             bias=1.0, scale=-1.0)
                        nc.vector.tensor_tensor(out=pt[:], in0=hdsl[:],
                                                in1=hhsl[:], op=ALU.mult)
                        for sw in range(SS):
                            xoff = ((1 + kd + sd) * XH * XWW + (1 + kh + sh) * XWW
                                    + (1 + kw + sw))
                            xap = mkap(xw, xoff, [[XH * XWW, D], [XWW, BH], [1, W]])
                            dst = at if sw == 0 else tt
                            nc.vector.tensor_tensor(out=dst[:], in0=xap,
                                                    in1=hw[:, sw, :], op=ALU.mult)
                            if sw > 0:
                                nc.vector.tensor_tensor(out=at[:], in0=at[:],
                                                        in1=tt[:], op=ALU.add)
                        if first:
                            nc.vector.tensor_tensor(out=cols[:], in0=pt[:], in1=at[:],
                                                    op=ALU.mult)
                            first = False
                        else:
                            nc.gpsimd.tensor_tensor(out=tt[:], in0=pt[:], in1=at[:],
                                                    op=ALU.mult)
                            nc.gpsimd.tensor_tensor(out=cols[:], in0=cols[:], in1=tt[:],
                                                    op=ALU.add)
                nc.sync.dma_start(
                    out=dmkap(colsd_dram[:], (b * K + k) * CH,
                              [[B * K * CH, P], [1, CH]]),
                    in_=cols[:])
            tc.strict_bb_all_engine_barrier()
            for hb in range(NB):
                ps2 = pspool.tile([64, 2048], F32, tag="eps")
                for k in range(K):
                    cr = wpool.tile([CPG, CH], F32, tag="colsr")
                    nc.sync.dma_start(
                        out=cr[:],
                        in_=dmkap(colsd_dram[:], hb * CPG * B * K * CH + (b * K + k) * CH,
                                  [[B * K * CH, CPG], [1, CH]]))
                    for i in range(4):
                        nc.tensor.matmul(ps2[:, i * 512:i * 512 + 448], wd_sb[:, k, :],
                                         cr[:, i * 448:(i + 1) * 448],
                                         start=(k == 0), stop=(k == K - 1))
                pot = wpool.tile([64, CH], F32, tag="pot")
                nc.vector.tensor_copy(out=pot[:], in_=mkap(ps2, 0, [[512, 4], [1, 448]]))
                nc.sync.dma_start(
                    out=dmkap(partial_dram[:], b * V + hb * CH, [[BV, 64], [1, CH]]),
                    in_=pot[:])


def ensure_consts(nc):
    for v in (2.0, -2.0, -1.0, 1e-5):
        key = (F32, v)
        if key not in nc.const_aps.aps:
            t = nc.alloc_sbuf_tensor(f"const-f32-{v}", [128, 1], F32)
            nc.gpsimd.memset(t.ap(), v)
            nc.const_aps.aps[key] = t.ap()


def bn_stats8(nc, tc, pool, sfx, src_dram, gamma_sb, beta_sb):
    """BN scale/shift for [CPG, BV] shard (band layout is irrelevant)."""
    sum_t = pool.tile([CPG, 1], F32, tag=f"bnsum{sfx}")
    sq_t = pool.tile([CPG, 1], F32, tag=f"bnsq{sfx}")
    t1 = pool.tile([CPG, 1], F32, tag=f"bnt1{sfx}")
    t2 = pool.tile([CPG, 1], F32, tag=f"bnt2{sfx}")
    with tc.tile_pool(name=f"bnstat{sfx}", bufs=1) as big:
        for i in range(NCHK):
            ht = big.tile([CPG, CSZ], F32, tag=f"bnh{sfx}")
            sqv = big.tile([CPG, CSZ], F32, tag=f"bnsqv{sfx}")
            nc.sync.dma_start(out=ht[:], in_=dmkap(src_dram, i * CSZ, [[BV, CPG], [1, CSZ]]))
            nc.vector.tensor_reduce(out=t1[:], in_=ht[:], axis=AX.X, op=ALU.add)
            nc.vector.tensor_tensor(out=sqv[:], in0=ht[:], in1=ht[:], op=ALU.mult)
            nc.vector.tensor_reduce(out=t2[:], in_=sqv[:], axis=AX.X, op=ALU.add)
            if i == 0:
                nc.vector.tensor_copy(out=sum_t[:], in_=t1[:])
                nc.vector.tensor_copy(out=sq_t[:], in_=t2[:])
            else:
                nc.vector.tensor_tensor(out=sum_t[:], in0=sum_t[:], in1=t1[:], op=ALU.add)
                nc.vector.tensor_tensor(out=sq_t[:], in0=sq_t[:], in1=t2[:], op=ALU.add)
    N = float(BV)
    scale = pool.tile([CPG, 1], F32, tag=f"bnscale{sfx}")
    shift = pool.tile([CPG, 1], F32, tag=f"bnshift{sfx}")
    mean = t1
    nc.vector.tensor_scalar(out=mean[:], in0=sum_t[:], scalar1=1.0 / N, scalar2=0.0,
                            op0=ALU.mult, op1=ALU.add)
    var = t2
    nc.vector.tensor_scalar(out=var[:], in0=sq_t[:], scalar1=1.0 / N, scalar2=0.0,
                            op0=ALU.mult, op1=ALU.add)
    msq = pool.tile([CPG, 1], F32, tag=f"bnmsq{sfx}")
    nc.vector.tensor_tensor(out=msq[:], in0=mean[:], in1=mean[:], op=ALU.mult)
    nc.vector.tensor_tensor(out=var[:], in0=var[:], in1=msq[:], op=ALU.subtract)
    rstd = pool.tile([CPG, 1], F32, tag=f"bnrstd{sfx}")
    nc.scalar.activation(out=rstd[:], in_=var[:], func=ACTF.Sqrt, bias=1e-5, scale=1.0)
    nc.vector.reciprocal(out=rstd[:], in_=rstd[:])
    nc.vector.tensor_tensor(out=scale[:], in0=gamma_sb[:], in1=rstd[:], op=ALU.mult)
    nc.vector.tensor_tensor(out=shift[:], in0=mean[:], in1=scale[:], op=ALU.mult)
    nc.vector.tensor_tensor(out=shift[:], in0=beta_sb[:], in1=shift[:], op=ALU.subtract)
    return scale, shift


NCHK = 4
CSZ = BV // NCHK


def deband_store(nc, src_tile, dst_dram, i):
    """Store band-layout chunk i of [CPG, CSZ] to canonical [CPG, BV] DRAM."""
    b, half = i // 2, i % 2
    for hbr in range(7):
        hb = half * 7 + hbr
        nc.sync.dma_start(
            out=dmkap(dst_dram, b * V + hb * BH * W, [[BV, CPG], [3136, D], [1, BH * W]]),
            in_=mkap(src_tile, hbr * CH, [[BH * W, D], [1, BH * W]]))


def band_load(nc, dst_tile, src_dram, i):
    """Load canonical [CPG, BV] DRAM into band-layout chunk i [CPG, CSZ]."""
    b, half = i // 2, i % 2
    for hbr in range(7):
        hb = half * 7 + hbr
        nc.sync.dma_start(
            out=mkap(dst_tile, hbr * CH, [[BH * W, D], [1, BH * W]]),
            in_=dmkap(src_dram, b * V + hb * BH * W, [[BV, CPG], [3136, D], [1, BH * W]]))


# ------------------------------------------------------------ program nc --
def build_fused(debug=False):
    nc = bass.Bass("TRN2", target_bir_lowering=False, num_devices=NCORES)
    ensure_consts(nc)
    px16_in = nc.declare_dram_parameter("px16", [1, NX16], F16, isOutput=False)
    pw16_in = nc.declare_dram_parameter("pw16", [1, NW16], F16, isOutput=False)
    pk32_in = nc.declare_dram_parameter("pk32", [1, N32], F32, isOutput=False)
    out_d = nc.declare_dram_parameter("out", [64, BV], F16, isOutput=True)
    og16_d = nc.dram_tensor("og16", [CPG, BV], F16)
    ofull16_d = nc.dram_tensor("ofull16", [64, BV], F16, addr_space="Shared")

    xs32_d = nc.dram_tensor("xs32", [CPG, BV], F32)
    xb16_d = nc.dram_tensor("xb16", [CPG, BV], F16)
    xfull16_d = nc.dram_tensor("xfull16", [64, BV], F16, addr_space="Shared")
    xfull_d = nc.dram_tensor("xfull", [64, BV], F32)
    xpad_d = nc.dram_tensor("xpad", [64, B * 10 * PLANE], F32)
    xw_d = nc.dram_tensor("xw", [CPG, B * XVOL], F32)
    xw2_d = nc.dram_tensor("xw2", [CPG, B * XVOL], F32)
    hpad_d = nc.dram_tensor("hpad", [64, B * 10 * PLANE], F32)
    off_d = nc.dram_tensor("offs", [OCG, B * D * 3136], F32)
    colsd_d = nc.dram_tensor("colsd", [P, B * K * CH], F32)
    part_d = nc.dram_tensor("part", [64, BV], F32)
    h1s_d = nc.dram_tensor("h1s", [CPG, BV], F32)
    h2s_d = nc.dram_tensor("h2s", [CPG, BV], F32)
    hg1_d = nc.dram_tensor("hg1", [CPG, BV], F32)
    hfull_d = nc.dram_tensor("hfull", [64, BV], F32, addr_space="Shared")

    dbg = {}
    if debug:
        for nm, shp in (("dxfull", [64, BV]), ("doff1", [OCG, BV]),
                        ("dpart1", [64, BV]), ("dh1s", [CPG, BV]),
                        ("dhg1", [CPG, BV]), ("dhfull", [64, BV]),
                        ("doff2", [OCG, BV]), ("dpart2", [64, BV]),
                        ("dh2s", [CPG, BV]), ("dxpad", [64, B * 10 * PLANE]),
                        ("dxw", [CPG, B * XVOL])):
            dbg[nm] = nc.declare_dram_parameter(nm, shp, F32, isOutput=True)

    def dump(nm, src, rows, total):
        if not debug:
            return
        nc.sync.dma_start(out=dmkap(dbg[nm][:], 0, [[total, rows], [1, total]]),
                          in_=dmkap(src[:], 0, [[total, rows], [1, total]]))

    with TileContext(nc) as tc:
        with tc.tile_pool(name="single", bufs=1) as sp:
            wt1_sb = sp.tile([64, K, OCG], F32, tag="wt1")
            wt2_sb = sp.tile([64, K, OCG], F32, tag="wt2")
            for woff, wsb, wtag in ((WT1_OFF, wt1_sb, "w16a"), (WT2_OFF, wt2_sb, "w16b")):
                w16 = sp.tile([64, K * OCG], F16, tag=wtag)
                nc.sync.dma_start(
                    out=w16[:],
                    in_=dmkap(pw16_in[:], woff, [[K * OCG, 64], [1, K * OCG]]))
                nc.vector.tensor_copy(out=wsb[:], in_=w16[:])
            bo1_sb = sp.tile([OCG, 1], F32, tag="bo1")
            nc.sync.dma_start(out=bo1_sb[:],
                              in_=dmkap(pk32_in[:], BO1_OFF, [[1, OCG], [1, 1]]))
            bo2_sb = sp.tile([OCG, 1], F32, tag="bo2")
            nc.sync.dma_start(out=bo2_sb[:],
                              in_=dmkap(pk32_in[:], BO2_OFF, [[1, OCG], [1, 1]]))
            wd1_sb = sp.tile([CPG, K, 64], F32, tag="wd1")
            nc.sync.dma_start(out=wd1_sb[:],
                              in_=dmkap(pk32_in[:], WD1_OFF, [[K * 64, CPG], [1, K * 64]]))
            wd2_sb = sp.tile([CPG, K, 64], F32, tag="wd2")
            nc.sync.dma_start(out=wd2_sb[:],
                              in_=dmkap(pk32_in[:], WD2_OFF, [[K * 64, CPG], [1, K * 64]]))
            gb_sb = sp.tile([CPG, 4], F32, tag="gb")
            nc.sync.dma_start(out=gb_sb[:],
                              in_=dmkap(pk32_in[:], GB_OFF, [[4, CPG], [1, 4]]))
            # zero all padded scratch volumes up front
            with tc.tile_pool(name="zpool", bufs=1) as zp:
                zero_sb = zp.tile([64, ZBLK], F32, tag="zsb")
                nc.vector.memset(zero_sb[:], 0.0)
                zero_dram(nc, zero_sb, xpad_d[:], 64, B * 10 * PLANE)
                zero_dram(nc, zero_sb, hpad_d[:], 64, B * 10 * PLANE)
                zero_dram(nc, zero_sb, xw_d[:], CPG, B * XVOL)
                zero_dram(nc, zero_sb, xw2_d[:], CPG, B * XVOL)

            # AllGather x slices (f16) -> xfull16, then cast passes
            nc.gpsimd.dma_start(
                out=xb16_d[:],
                in_=dmkap(px16_in[:], XS_OFF, [[BV, CPG], [1, BV]]))
            nc.gpsimd.collective_compute(
                "AllGather", ALU.bypass, replica_groups=RG,
                ins=[xb16_d[:].opt()], outs=[xfull16_d[:].opt()])
            # xs cast to f32 (local slice; overlaps with the collective)
            with tc.tile_pool(name="xcast", bufs=2) as cp:
                for i in range(NCHK):
                    a16 = cp.tile([CPG, CSZ], F16, tag="a16")
                    nc.sync.dma_start(
                        out=a16[:],
                        in_=dmkap(px16_in[:], XS_OFF + i * CSZ, [[BV, CPG], [1, CSZ]]))
                    a32 = cp.tile([CPG, CSZ], F32, tag="a32")
                    nc.vector.tensor_copy(out=a32[:], in_=a16[:])
                    nc.sync.dma_start(
                        out=dmkap(xs32_d[:], i * CSZ, [[BV, CPG], [1, CSZ]]),
                        in_=a32[:])
            tc.strict_bb_all_engine_barrier()
            # xfull cast to f32
            with tc.tile_pool(name="xfcast", bufs=2) as cp:
                for i in range(NCHK):
                    a16 = cp.tile([64, CSZ], F16, tag="b16")
                    nc.sync.dma_start(
                        out=a16[:],
                        in_=dmkap(xfull16_d[:], i * CSZ, [[BV, 64], [1, CSZ]]))
                    a32 = cp.tile([64, CSZ], F32, tag="b32")
                    nc.vector.tensor_copy(out=a32[:], in_=a16[:])
                    nc.sync.dma_start(
                        out=dmkap(xfull_d[:], i * CSZ, [[BV, 64], [1, CSZ]]),
                        in_=a32[:])
            tc.strict_bb_all_engine_barrier()
            dump("dxfull", xfull_d, 64, BV)

            # interior fills: xpad <- xfull, xw <- xs32
            for b in range(B):
                for d in range(D):
                    nc.sync.dma_start(
                        out=dmkap(xpad_d[:], (b * 10 + d + 1) * PLANE + 59,
                                  [[B * 10 * PLANE, 64], [58, 56], [1, 56]]),
                        in_=dmkap(xfull_d[:], b * V + d * 3136,
                                  [[BV, 64], [56, 56], [1, 56]]))
                    nc.sync.dma_start(
                        out=dmkap(xw_d[:], b * XVOL + (d + 3) * 3844 + 3 * 62 + 3,
                                  [[B * XVOL, CPG], [62, 56], [1, 56]]),
                        in_=dmkap(xs32_d[:], b * V + d * 3136,
                                  [[BV, CPG], [56, 56], [1, 56]]))
            tc.strict_bb_all_engine_barrier()

            dump("dxpad", xpad_d, 64, B * 10 * PLANE)
            dump("dxw", xw_d, CPG, B * XVOL)
            tc.strict_bb_all_engine_barrier()
            # ---- layer 1
            conv_phase(nc, tc, "1", xpad_d, wt1_sb, bo1_sb, off_d)
            tc.strict_bb_all_engine_barrier()
            dump("doff1", off_d, OCG, BV)
            dense_phase(nc, tc, "1", xw_d, off_d, wd1_sb, part_d, colsd_d)
            tc.strict_bb_all_engine_barrier()
            dump("dpart1", part_d, 64, BV)
            nc.gpsimd.collective_compute(
                "ReduceScatter", ALU.add, replica_groups=RG,
                ins=[part_d[:].opt()], outs=[h1s_d[:].opt()])
            tc.strict_bb_all_engine_barrier()
            dump("dh1s", h1s_d, CPG, BV)

            # ---- BN1 + relu -> hg1 (canonical), then AllGather -> hfull
            with tc.tile_pool(name="bn1p", bufs=1) as pool:
                scale, shift = bn_stats8(nc, tc, pool, "a", h1s_d[:], gb_sb[:, 0:1],
                                         gb_sb[:, 1:2])
                with tc.tile_pool(name="bn1ap", bufs=2) as apool:
                    for i in range(NCHK):
                        ht = apool.tile([CPG, CSZ], F32, tag="bnh1")
                        nc.sync.dma_start(
                            out=ht[:], in_=dmkap(h1s_d[:], i * CSZ, [[BV, CPG], [1, CSZ]]))
                        nc.scalar.activation(out=ht[:], in_=ht[:], func=ACTF.Relu,
                                             bias=shift[:], scale=scale[:])
                        deband_store(nc, ht, hg1_d[:], i)
            tc.strict_bb_all_engine_barrier()
            dump("dhg1", hg1_d, CPG, BV)
            nc.gpsimd.collective_compute(
                "AllGather", ALU.bypass, replica_groups=RG,
                ins=[hg1_d[:].opt()], outs=[hfull_d[:].opt()])
            tc.strict_bb_all_engine_barrier()
            dump("dhfull", hfull_d, 64, BV)

            # interior fills: hpad <- hfull, xw2 <- hg1
            for b in range(B):
                for d in range(D):
                    nc.sync.dma_start(
                        out=dmkap(hpad_d[:], (b * 10 + d + 1) * PLANE + 59,
                                  [[B * 10 * PLANE, 64], [58, 56], [1, 56]]),
                        in_=dmkap(hfull_d[:], b * V + d * 3136,
                                  [[BV, 64], [56, 56], [1, 56]]))
                    nc.sync.dma_start(
                        out=dmkap(xw2_d[:], b * XVOL + (d + 3) * 3844 + 3 * 62 + 3,
                                  [[B * XVOL, CPG], [62, 56], [1, 56]]),
                        in_=dmkap(hg1_d[:], b * V + d * 3136,
                                  [[BV, CPG], [56, 56], [1, 56]]))
            tc.strict_bb_all_engine_barrier()

            # ---- layer 2
            conv_phase(nc, tc, "2", hpad_d, wt2_sb, bo2_sb, off_d)
            tc.strict_bb_all_engine_barrier()
            dump("doff2", off_d, OCG, BV)
            dense_phase(nc, tc, "2", xw2_d, off_d, wd2_sb, part_d, colsd_d)
            tc.strict_bb_all_engine_barrier()
            dump("dpart2", part_d, 64, BV)
            nc.gpsimd.collective_compute(
                "ReduceScatter", ALU.add, replica_groups=RG,
                ins=[part_d[:].opt()], outs=[h2s_d[:].opt()])
            tc.strict_bb_all_engine_barrier()
            dump("dh2s", h2s_d, CPG, BV)

            # ---- BN2 + residual + relu -> out (canonical)
            with tc.tile_pool(name="bn2p", bufs=1) as pool:
                scale, shift = bn_stats8(nc, tc, pool, "b", h2s_d[:], gb_sb[:, 2:3],
                                         gb_sb[:, 3:4])
                with tc.tile_pool(name="bn2ap", bufs=1) as apool:
                    for i in range(NCHK):
                        ht = apool.tile([CPG, CSZ], F32, tag="bnh2")
                        rt = apool.tile([CPG, CSZ], F32, tag="bnr2")
                        nc.sync.dma_start(
                            out=ht[:], in_=dmkap(h2s_d[:], i * CSZ, [[BV, CPG], [1, CSZ]]))
                        band_load(nc, rt, xs32_d[:], i)
                        nc.vector.tensor_tensor(out=ht[:], in0=ht[:],
                                                in1=mkap(scale, 0, [[0, CSZ]]), op=ALU.mult)
                        nc.vector.tensor_tensor(out=ht[:], in0=ht[:],
                                                in1=mkap(shift, 0, [[0, CSZ]]), op=ALU.add)
                        nc.vector.tensor_tensor(out=ht[:], in0=ht[:], in1=rt[:], op=ALU.add)
                        ht16 = apool.tile([CPG, CSZ], F16, tag="bnh2c")
                        nc.vector.tensor_scalar(out=ht16[:], in0=ht[:], scalar1=0.0,
                                                scalar2=0.0, op0=ALU.max, op1=ALU.add)
                        deband_store(nc, ht16, og16_d[:], i)
            # replicate the output on every core so the host fetches 1 shard
            tc.strict_bb_all_engine_barrier()
            nc.gpsimd.collective_compute(
                "AllGather", ALU.bypass, replica_groups=RG,
                ins=[og16_d[:].opt()], outs=[ofull16_d[:].opt()])
            tc.strict_bb_all_engine_barrier()
            nc.sync.dma_start(out=out_d[:], in_=ofull16_d[:])
    return nc


# ---------------------------------------------------------- cached runner --
class Runner:
    """jit(shard_map(bass_exec)) built once; later calls only move data."""

    def __init__(self, nc, n_cores=NCORES):
        _b2j.install_neuronx_cc_hook()
        self.n_cores = n_cores
        partition_name = nc.partition_id_tensor.name if nc.partition_id_tensor else None
        in_names, out_names, out_avals, zero_shapes = [], [], [], []
        for alloc in nc.m.functions[0].allocations:
            if not isinstance(alloc, mybir.MemoryLocationSet):
                continue
            name = alloc.memorylocations[0].name
            if alloc.kind == "ExternalInput":
                if name != partition_name:
                    in_names.append(name)
            elif alloc.kind == "ExternalOutput":
                shape = tuple(alloc.tensor_shape)
                dtype = mybir.dt.np(alloc.dtype)
                out_names.append(name)
                out_avals.append(jax.core.ShapedArray(shape, dtype))
                zero_shapes.append((shape, dtype))
        self.n_params = len(in_names)
        self.in_names = list(in_names)
        self.out_names = out_names
        self.out_avals = out_avals
        all_in_names = list(in_names)
        if partition_name is not None:
            all_in_names.append(partition_name)

        def _body(*args):
            operands = list(args)
            if partition_name is not None:
                operands.append(_b2j.partition_id_tensor())
            outs = _b2j._bass_exec_p.bind(
                *operands,
                out_avals=tuple(out_avals),
                in_names=tuple(all_in_names),
                out_names=tuple(out_names),
                lowering_input_output_aliases=(),
                sim_require_finite=True,
                sim_require_nnan=True,
                nc=nc,
            )
            return tuple(outs)

        devices = jax.devices()[:n_cores]
        assert len(devices) == n_cores
        self.mesh = Mesh(np.asarray(devices), ("core",))
        self.fn = jax.jit(
            shard_map(_body, mesh=self.mesh,
                      in_specs=(PartitionSpec("core"),) * self.n_params,
                      out_specs=(PartitionSpec("core"),) * len(out_names),
                      check_rep=False),
            keep_unused=True,
        )

    def __call__(self, in_maps):
        concat_in = [
            np.concatenate([np.asarray(m[name]) for m in in_maps], axis=0)
            for name in self.in_names
        ]
        out_arrs = self.fn(*concat_in)
        outs = [np.asarray(a) for a in out_arrs]
        return [
            {
                name: outs[i].reshape(self.n_cores, *self.out_avals[i].shape)[c]
                for i, name in enumerate(self.out_names)
            }
            for c in range(self.n_cores)
        ]


_RUNNER = None


def _get_runner():
    global _RUNNER
    if _RUNNER is None:
        _RUNNER = Runner(build_fused())
    return _RUNNER


# ----------------------------------------------------------------- kernel --
WKEYS = ("w_off1", "b_off1", "w_dc1", "w_off2", "b_off2", "w_dc2",
         "gamma1", "beta1", "gamma2", "beta2")


def pack_x(inputs):
    x = np.asarray(inputs["x"], dtype=np.float32)
    px16 = np.empty((G, NX16), np.float16)
    for g in range(G):
        px16[g] = x[:, g * CPG:(g + 1) * CPG].transpose(1, 0, 2, 3, 4).reshape(-1)
    return px16


def pack_weights(inputs):

    def wslices(w_off, b_off, w_dc):
        wts, bs, wds = [], [], []
        w_off = np.asarray(w_off, np.float32).reshape(G * OCG, 64, K)
        w_dc = np.asarray(w_dc, np.float32).reshape(64, G, CPG, K)
        b_off = np.asarray(b_off, np.float32)
        for g in range(G):
            wts.append(np.ascontiguousarray(
                w_off[g * OCG:(g + 1) * OCG].transpose(1, 2, 0)).reshape(64, -1))
            bs.append(np.ascontiguousarray(b_off[g * OCG:(g + 1) * OCG]).reshape(OCG, 1))
            wds.append(np.ascontiguousarray(
                w_dc[:, g].transpose(1, 2, 0)).reshape(CPG, -1))
        return wts, bs, wds

    wt1, bo1, wd1 = wslices(inputs["w_off1"], inputs["b_off1"], inputs["w_dc1"])
    wt2, bo2, wd2 = wslices(inputs["w_off2"], inputs["b_off2"], inputs["w_dc2"])
    g1 = np.asarray(inputs["gamma1"], np.float32)
    b1 = np.asarray(inputs["beta1"], np.float32)
    g2 = np.asarray(inputs["gamma2"], np.float32)
    b2 = np.asarray(inputs["beta2"], np.float32)

    pw16 = np.empty((G, NW16), np.float16)
    pk32 = np.empty((G, N32), np.float32)
    for g in range(G):
        sl = slice(g * CPG, (g + 1) * CPG)
        gb = np.stack([g1[sl], b1[sl], g2[sl], b2[sl]], axis=1).astype(np.float32)
        pw16[g, WT1_OFF:WT1_OFF + 64 * K * OCG] = wt1[g].reshape(-1)
        pw16[g, WT2_OFF:WT2_OFF + 64 * K * OCG] = wt2[g].reshape(-1)
        pk32[g, BO1_OFF:BO1_OFF + OCG] = bo1[g].reshape(-1)
        pk32[g, BO2_OFF:BO2_OFF + OCG] = bo2[g].reshape(-1)
        pk32[g, WD1_OFF:WD1_OFF + CPG * K * 64] = wd1[g].reshape(-1)
        pk32[g, WD2_OFF:WD2_OFF + CPG * K * 64] = wd2[g].reshape(-1)
        pk32[g, GB_OFF:GB_OFF + CPG * 4] = gb.reshape(-1)
    return pw16, pk32


def make_inmaps(inputs):
    # kept for debug tooling: full (px16, pw16, pk32) pack
    px16 = pack_x(inputs)
    pw16, pk32 = pack_weights(inputs)
    return px16, pw16, pk32


_WCACHE = {"raw": None, "dev": None}


def kernel(**inputs):
    runner = _get_runner()
    px16 = pack_x(inputs)
    # weights live on device across calls; re-pack/re-ship only on change
    raw = [np.asarray(inputs[k], np.float32) for k in WKEYS]
    if (_WCACHE["dev"] is None
            or not all(np.array_equal(a, b)
                       for a, b in zip(raw, _WCACHE["raw"]))):
        pw16, pk32 = pack_weights(inputs)
        shard = NamedSharding(runner.mesh, PartitionSpec("core"))
        dev = (jax.device_put(pw16, shard), jax.device_put(pk32, shard))
        jax.block_until_ready(dev)
        _WCACHE.update(raw=raw, dev=dev)
    dev_pw16, dev_pk32 = _WCACHE["dev"]
    args = {"px16": px16, "pw16": dev_pw16, "pk32": dev_pk32}
    out_arrs = runner.fn(*[args[n] for n in runner.in_names])
    oi = runner.out_names.index("out")
    # output is AllGather-replicated across cores; fetch a single shard
    out = np.asarray(out_arrs[oi].addressable_shards[0].data)  # [64, BV] f16
    return np.ascontiguousarray(
        out.reshape(64, B, D, H, W).transpose(1, 0, 2, 3, 4)).astype(np.float32)
